# revision 14
# baseline (speedup 1.0000x reference)
"""Trainium2 Bass kernel for nn_LiquidNeuralNetwork (v2).

Strategy: data-parallel over batch (8 cores x 64). Per core, a fully on-chip
recurrence over T=512 steps with the two LTC layers run as TWO INDEPENDENT
INSTRUCTION STREAMS (layer0 at tick t, layer1 at tick t-1), interleaved at
RK4-stage granularity so the tensor-engine matmuls of one stream hide the
scalar/vector latency of the other.

Per-layer tiles are [128 part = h%128, free = m*64 + b] (m = h//128 output
half, b = batch-in-core). The gate sigmoid(tanh(u)) is replaced by the fitted
a*tanh(b*u)+0.5 (max abs err 6.7e-4) so each RK4 stage costs one ScalarE tanh
plus one fused DVE (t*a+0.5)*R multiply. The LayerNorm rsqrt runs as 4 custom
DVE row ops (linear-seed + reciprocal_approx_fast + 2 Newton steps in w-form).
RK4 P-assembly and dr-deltas run on the otherwise-idle GpSimd engine.
"""

import os
import sys
import numpy as np

sys.path.insert(0, "/opt/trn_rl_repo")

B, T, FS, FC, H, L = 512, 512, 64, 32, 256, 2
LN_EPS = 1e-5
NCORES = 8
BC = B // NCORES       # 64 batch per core
UNROLL = 8
N_H = 256.0

# gate fit: sigmoid(tanh(u)) ~= GATE_A * tanh(GATE_B * u) + 0.5
GATE_A, GATE_B = 0.230386, 1.072557

# rsqrt(vv) over observed vv range [1.0e5, 1.7e6] with 4x safety margin
VLO, VHI = 2.5e4, 6.8e6
_VC = float(np.sqrt(VLO * VHI))
SEED_B = 0.5 / float(np.sqrt(_VC))
SEED_A = 1.0 / (4.0 * SEED_B)
# w = SEED_B*(N*S2 + 36*N^2*eps - S1^2) + SEED_A ~= sqrt(vv)
VVW_C0 = SEED_B * N_H
VVW_C1 = SEED_A + SEED_B * 36.0 * N_H * N_H * LN_EPS
VVW_C2 = SEED_B
# Newton in w-form: y' = (1.5 - w*y^2*(0.5/b) + (0.5a/b)*y^2) * y
NRW_C0 = 1.5
NRW_C1 = 0.5 / SEED_B
NRW_C2 = 0.5 * SEED_A / SEED_B

USE_GPSIMD = os.environ.get("LNN_NO_GPSIMD") != "1"


def softplus_np(x):
    return np.log1p(np.exp(-np.abs(x))) + np.maximum(x, 0)


# ---------------------------------------------------------------------------
# Custom DVE ops
# ---------------------------------------------------------------------------

_OPS_CACHE = {}


def _get_custom_ops():
    if _OPS_CACHE:
        return _OPS_CACHE
    from concourse.dve_spec import Spec, Src0, Src1, C0, C1, C2, lower, sq
    from concourse.dve_spec import _has_src1
    from concourse.dve_uop import DveOpSpec
    from concourse import dve_ops

    _m = sq(Src1)
    defs = {
        # out = (in0*s0 + s1) * in1   -- gate affine folded into the R-multiply
        "GATE_MUL_LNN": (
            (Src0 * C0 + C1) * Src1,
            lambda in0, in1, s0, s1, imm2: (
                (in0.astype(np.float32) * s0 + s1) * in1
            ).astype(np.float32),
        ),
        # out = (in0*s0 + s1) - in1^2 * imm2   -- w = b*(N*S2 + c - S1^2) + a
        "VV_W_LNN": (
            (Src0 * C0 + C1) - sq(Src1) * C2,
            lambda in0, in1, s0, s1, imm2: (
                (in0.astype(np.float32) * s0 + s1)
                - np.square(in1.astype(np.float32)) * imm2
            ).astype(np.float32),
        ),
        # out = (s0 - in0*in1^2*s1 + imm2*in1^2) * in1  -- Newton step, w-form
        "NR_W_LNN": (
            (C0 - Src0 * _m * C1 + C2 * _m) * Src1,
            lambda in0, in1, s0, s1, imm2: (
                (
                    s0
                    - in0.astype(np.float32) * np.square(in1.astype(np.float32)) * s1
                    + imm2 * np.square(in1.astype(np.float32))
                )
                * in1
            ).astype(np.float32),
        ),
    }
    for name, (body, ref) in defs.items():
        if name in dve_ops._SUB_OPCODE_FOR_NAME:
            _OPS_CACHE[name] = next(o for o in dve_ops.OPS if o.name == name)
            continue
        spec = Spec(body=body, reference=ref)
        opcode = dve_ops._CUSTOM_DVE_ROW_BASE + len(dve_ops.OPS)
        shas = {}
        for ver in ("v3", "v4"):
            shas[ver] = DveOpSpec(
                name=name,
                opcode=opcode,
                uops=lower(spec, ver=ver),
                rd1_en=_has_src1(spec),
            ).sha(ver)
        op = dve_ops.DveOp(name, spec, subdim=False, uops_sha=shas)
        dve_ops.OPS.append(op)
        dve_ops._SUB_OPCODE_FOR_NAME[name] = opcode
        dve_ops.CUSTOM_DVE_SPECS[name] = spec
        _OPS_CACHE[name] = op
    return _OPS_CACHE


# ---------------------------------------------------------------------------
# Bass module builder
# ---------------------------------------------------------------------------

def build_module(T_run=T, unroll=UNROLL):
    import concourse.bass as bass
    import concourse.mybir as mybir
    from concourse import tile, bacc
    from concourse.bass import ds

    ops = _get_custom_ops()
    GATE_MUL = ops["GATE_MUL_LNN"]
    VV_W = ops["VV_W_LNN"]
    NR_W = ops["NR_W_LNN"]

    f32 = mybir.dt.float32
    bf16 = mybir.dt.bfloat16
    AF = mybir.ActivationFunctionType
    OP = mybir.AluOpType

    nc = bacc.Bacc(None, target_bir_lowering=False)

    xcat = nc.declare_dram_parameter("xcat", [T_run, 97, BC], bf16, isOutput=False)
    wdecl = {}

    def wparam(name, shape):
        wdecl[name] = nc.declare_dram_parameter(name, shape, bf16, isOutput=False)
        return wdecl[name]

    wparam("g_x0", [97, 256])
    wparam("in_x0", [96, 256])
    wparam("g_h0", [128, 512])
    wparam("g_h0h", [128, 512])
    wparam("rec0", [128, 512])
    wparam("rec0h", [128, 512])
    wparam("dneg0", [128, 256])
    wparam("dneg0h", [128, 256])
    wparam("g_x1", [128, 512])
    wparam("bg1row", [1, 256])
    wparam("in_x1", [128, 512])
    wparam("g_h1", [128, 512])
    wparam("g_h1h", [128, 512])
    wparam("rec1", [128, 512])
    wparam("rec1h", [128, 512])
    wparam("dneg1", [128, 256])
    wparam("dneg1h", [128, 256])
    wparam("lnT", [2, 512])      # rows: [-lng ; lnb], col blocks (2l+m)*128
    wparam("lngN", [1, 512])     # lng * N
    wparam("ones_red", [128, 1])
    wparam("ones2", [2, 64])
    hout = nc.declare_dram_parameter("hout", [128, 128], bf16, isOutput=True)

    # slots 2..505 in the hw loop; 0..1 head; 506..513 tail
    LOOP_LO, LOOP_HI = 2, 506
    assert (LOOP_HI - LOOP_LO) % unroll == 0

    with tile.TileContext(nc) as tc:
        from contextlib import ExitStack
        with ExitStack() as ctx:
            singles = ctx.enter_context(tc.tile_pool(name="singles", bufs=1))
            xc_pool = ctx.enter_context(tc.tile_pool(name="xc", bufs=12))
            # PSUM: one bank each for G/R/V per layer; bc+stats share a bank.
            gps = [ctx.enter_context(tc.tile_pool(name=f"g{l}ps", bufs=1, space="PSUM")) for l in range(L)]
            rps = [ctx.enter_context(tc.tile_pool(name=f"r{l}ps", bufs=1, space="PSUM")) for l in range(L)]
            vps = [ctx.enter_context(tc.tile_pool(name=f"v{l}ps", bufs=1, space="PSUM")) for l in range(L)]
            tailps = [ctx.enter_context(tc.tile_pool(name=f"tail{l}ps", bufs=1, space="PSUM")) for l in range(L)]

            # ---- resident weights ---------------------------------------
            W = {}
            for name, dram in wdecl.items():
                t_ = singles.tile(list(dram.shape), bf16, name=name, tag=name)
                nc.sync.dma_start(t_[:], dram[:])
                W[name] = t_

            rings = [[singles.tile([128, 128], bf16, name=f"ring{l}_{i}", tag=f"ring{l}_{i}")
                      for i in range(4)] for l in range(L)]
            hz = singles.tile([128, 128], bf16)
            nc.vector.memset(hz[:], 0.0)
            nc.vector.memset(rings[1][3][:], 0.0)   # h1[-1] = 0
            onesr = singles.tile([1, BC], bf16)
            nc.vector.memset(onesr[:], 1.0)
            trhs = [singles.tile([2, BC], bf16, name=f"trhs{l}", tag=f"trhs{l}") for l in range(L)]
            for l in range(L):
                nc.sync.dma_start(trhs[l][:, :], wdecl["ones2"][:, :])

            # static per-stream work tiles (fixed names -> the software
            # pipeline can reference them across the hw-loop back edge)
            def mk(l, nm, dt):
                return singles.tile([128, 128], dt, name=f"{nm}{l}", tag=f"{nm}{l}")

            ST = []
            for l in range(L):
                d = dict(
                    tg=mk(l, "tg", f32), mm=mk(l, "mm", f32),
                    k1=mk(l, "k1", bf16), k2=mk(l, "k2", bf16),
                    k3=mk(l, "k3", bf16),
                    dr2=mk(l, "dr2", bf16), dr3=mk(l, "dr3", bf16),
                    c2=mk(l, "c2", f32), c3=mk(l, "c3", f32),
                    cP=mk(l, "cP", f32),
                    e1=mk(l, "e1", f32), e2=mk(l, "e2", f32),
                    e3=mk(l, "e3", f32), e4=mk(l, "e4", f32),
                    h2=mk(l, "h2", f32), h6=mk(l, "h6", f32),
                    P=mk(l, "P", bf16), P2=mk(l, "P2", bf16),
                    z=mk(l, "z", f32), z2=mk(l, "z2", f32),
                )
                for nm in ("sx", "w", "y0", "y1"):
                    d[nm] = singles.tile([1, BC], f32, name=nm + str(l), tag=nm + str(l))
                d["y2"] = singles.tile([1, BC], bf16, name=f"y2{l}", tag=f"y2{l}")
                d["G"] = gps[l].tile([128, 128], f32, name=f"G{l}", tag="G")
                d["R"] = rps[l].tile([128, 128], f32, name=f"R{l}", tag="R")
                d["V"] = vps[l].tile([128, 128], f32, name=f"V{l}", tag="V")
                d["tail"] = tailps[l].tile([128, 512], f32, name=f"tail{l}", tag="tail")
                ST.append(d)

            def wt(name, kt, m):
                return W[name][:, kt * 256 + m * 128: kt * 256 + (m + 1) * 128]

            def dneg(l, half, m):
                nm = f"dneg{l}" + ("h" if half else "")
                return W[nm][:, m * 128:(m + 1) * 128]

            eng2 = nc.gpsimd if USE_GPSIMD else nc.vector

            def g_sub(out, a, b):  # out = a - b  (SBUF-only operands)
                if USE_GPSIMD:
                    eng2.tensor_sub(out, a, b)
                else:
                    nc.vector.scalar_tensor_tensor(out=out, in0=b, scalar=-1.0,
                                                   in1=a, op0=OP.mult, op1=OP.add)

            def g_add(out, a, b):
                if USE_GPSIMD:
                    eng2.tensor_add(out, a, b)
                else:
                    nc.vector.scalar_tensor_tensor(out=out, in0=a, scalar=0.0,
                                                   in1=b, op0=OP.add, op1=OP.add)

            def g_mul(out, a, b):
                if USE_GPSIMD:
                    eng2.tensor_mul(out, a, b)
                else:
                    nc.vector.scalar_tensor_tensor(out=out, in0=a, scalar=0.0,
                                                   in1=b, op0=OP.add, op1=OP.mult)

            # per-stream bookkeeping (mm-group first flags survive one tick)
            first_flags = [{}, {}]

            def emit_stage(l, s, xc=None, x1=None, h_self=None):
                """RK4 stage s for layer l: G MMs, gate tanh, R/V MMs, fused
                gate multiply, k, and the next stage's dr."""
                d = ST[l]
                first = first_flags[l]

                def Gc(m):
                    return d["G"][:, m * 64:(m + 1) * 64]

                def Rc(m):
                    return d["R"][:, m * 64:(m + 1) * 64]

                def Vc(m):
                    return d["V"][:, m * 64:(m + 1) * 64]

                def bmm(bank, out_ap, lhsT, rhs, last=False):
                    st_ = bank not in first
                    first[bank] = True
                    nc.tensor.matmul(out_ap, lhsT, rhs, start=st_, stop=last,
                                     skip_group_check=True)

                dr = {1: None, 2: d["k1"], 3: d["dr2"], 4: d["dr3"]}[s]
                if s == 1:
                    first.clear()
                    if l == 0:
                        for m in range(2):
                            bmm("G", Gc(m), W["g_x0"][:, m * 128:(m + 1) * 128], xc[:, :])
                            for kt in range(2):
                                bmm("G", Gc(m), wt("g_h0", kt, m), h_self[:, kt * 64:(kt + 1) * 64])
                    else:
                        for m in range(2):
                            for kt in range(2):
                                bmm("G", Gc(m), wt("g_x1", kt, m), x1[:, kt * 64:(kt + 1) * 64])
                            bmm("G", Gc(m), W["bg1row"][:, m * 128:(m + 1) * 128], onesr[:, :])
                            for kt in range(2):
                                bmm("G", Gc(m), wt("g_h1", kt, m), h_self[:, kt * 64:(kt + 1) * 64])
                    nc.scalar.activation(d["tg"][:, :], d["G"][:, :], AF.Tanh, scale=GATE_B)
                    for m in range(2):
                        for kt in range(2):
                            bmm("R", Rc(m), wt(f"rec{l}", kt, m), h_self[:, kt * 64:(kt + 1) * 64])
                    if l == 0:
                        for m in range(2):
                            bmm("V", Vc(m), W["in_x0"][:, m * 128:(m + 1) * 128], xc[0:96, :])
                            bmm("V", Vc(m), dneg(0, False, m), h_self[:, m * 64:(m + 1) * 64])
                    else:
                        for m in range(2):
                            for kt in range(2):
                                bmm("V", Vc(m), wt("in_x1", kt, m), x1[:, kt * 64:(kt + 1) * 64])
                            bmm("V", Vc(m), dneg(1, False, m), h_self[:, m * 64:(m + 1) * 64])
                else:
                    half = s in (2, 3)
                    sfx = "h" if half else ""
                    last = s == 4
                    for m in range(2):
                        for kt in range(2):
                            bmm("G", Gc(m), wt(f"g_h{l}{sfx}", kt, m),
                                dr[:, kt * 64:(kt + 1) * 64], last=last and m == 1 and kt == 1)
                    nc.scalar.activation(d["tg"][:, :], d["G"][:, :], AF.Tanh, scale=GATE_B)
                    for m in range(2):
                        for kt in range(2):
                            bmm("R", Rc(m), wt(f"rec{l}{sfx}", kt, m),
                                dr[:, kt * 64:(kt + 1) * 64], last=last and m == 1 and kt == 1)
                    for m in range(2):
                        bmm("V", Vc(m), dneg(l, half, m),
                            dr[:, m * 64:(m + 1) * 64], last=last and m == 1)
                if s == 2:
                    # c2 = V - k1 (off critical path, runs under the tanh)
                    nc.vector.scalar_tensor_tensor(out=d["c2"][:, :], in0=d["k1"][:, :],
                                                   scalar=-1.0, in1=d["V"][:, :],
                                                   op0=OP.mult, op1=OP.add)
                elif s == 3:
                    nc.vector.scalar_tensor_tensor(out=d["c3"][:, :], in0=d["k2"][:, :],
                                                   scalar=-0.5, in1=d["V"][:, :],
                                                   op0=OP.mult, op1=OP.add)
                nc.vector._custom_dve(GATE_MUL, out=d["mm"][:, :], in0=d["tg"][:, :],
                                      in1=d["R"][:, :], s0=GATE_A, s1=0.5)
                if s == 1:
                    nc.vector.scalar_tensor_tensor(out=d["k1"][:, :], in0=d["mm"][:, :],
                                                   scalar=0.0, in1=d["V"][:, :],
                                                   op0=OP.add, op1=OP.add)
                elif s == 2:
                    nc.vector.scalar_tensor_tensor(out=d["dr2"][:, :], in0=d["mm"][:, :],
                                                   scalar=0.0, in1=d["c2"][:, :],
                                                   op0=OP.add, op1=OP.add)
                    nc.vector.scalar_tensor_tensor(out=d["k2"][:, :], in0=d["mm"][:, :],
                                                   scalar=0.0, in1=d["V"][:, :],
                                                   op0=OP.add, op1=OP.add)
                elif s == 3:
                    nc.vector.scalar_tensor_tensor(out=d["dr3"][:, :], in0=d["mm"][:, :],
                                                   scalar=0.0, in1=d["c3"][:, :],
                                                   op0=OP.add, op1=OP.add)
                    nc.vector.scalar_tensor_tensor(out=d["k3"][:, :], in0=d["mm"][:, :],
                                                   scalar=0.0, in1=d["V"][:, :],
                                                   op0=OP.add, op1=OP.add)
                    g_add(d["e1"][:, :], d["k2"][:, :], d["k3"][:, :])
                else:
                    # P-prefix on GpSimd during the s4 matmul burst
                    g_add(d["e2"][:, :], d["e1"][:, :], d["e1"][:, :])
                    g_add(d["e3"][:, :], d["e2"][:, :], d["k1"][:, :])
                    g_add(d["h2"][:, :], h_self[:, :], h_self[:, :])
                    g_add(d["h6"][:, :], d["h2"][:, :], d["h2"][:, :])
                    g_add(d["h6"][:, :], d["h6"][:, :], d["h2"][:, :])
                    g_add(d["e4"][:, :], d["e3"][:, :], d["h6"][:, :])
                    nc.vector.scalar_tensor_tensor(out=d["cP"][:, :], in0=d["e4"][:, :],
                                                   scalar=0.0, in1=d["V"][:, :],
                                                   op0=OP.add, op1=OP.add)
                    nc.vector.scalar_tensor_tensor(out=d["P"][:, :], in0=d["mm"][:, :],
                                                   scalar=0.0, in1=d["cP"][:, :],
                                                   op0=OP.add, op1=OP.add)

            def emit_tail_a(l, h_self):
                d = ST[l]
                tl = d["tail"]
                nc.tensor.matmul(tl[0:1, 256:320], W["ones_red"][:, :], d["P"][:, 0:64],
                                 start=True, stop=False, skip_group_check=True)
                nc.tensor.matmul(tl[0:1, 256:320], W["ones_red"][:, :], d["P"][:, 64:128],
                                 start=False, stop=True, skip_group_check=True)
                nc.scalar.activation(d["sx"][:, :], tl[0:1, 256:320], AF.Copy)
                nc.scalar.activation(d["P2"][:, :], d["P"][:, :], AF.Square)
                nc.tensor.matmul(tl[0:1, 320:384], W["ones_red"][:, :], d["P2"][:, 0:64],
                                 start=True, stop=False, skip_group_check=True)
                nc.tensor.matmul(tl[0:1, 320:384], W["ones_red"][:, :], d["P2"][:, 64:128],
                                 start=False, stop=True, skip_group_check=True)

            def emit_tail_b(l, h_next):
                d = ST[l]
                tl = d["tail"]
                s2_ap = tl[0:1, 320:384]
                nc.vector._custom_dve(VV_W, out=d["w"][:, :], in0=s2_ap,
                                      in1=d["sx"][:, :], s0=VVW_C0, s1=VVW_C1, imm2=VVW_C2)
                nc.vector.reciprocal_approx_fast(out=d["y0"][:, :], in_=d["w"][:, :])
                nc.vector._custom_dve(NR_W, out=d["y1"][:, :], in0=d["w"][:, :], in1=d["y0"][:, :],
                                      s0=NRW_C0, s1=NRW_C1, imm2=NRW_C2)
                nc.vector._custom_dve(NR_W, out=d["y2"][:, :], in0=d["w"][:, :], in1=d["y1"][:, :],
                                      s0=NRW_C0, s1=NRW_C1, imm2=NRW_C2)
                g_mul(trhs[l][0:1, :], d["sx"][:, :], d["y2"][:, :])
                for m in range(2):
                    lq = (2 * l + m) * 128
                    nc.tensor.matmul(tl[:, m * 64:(m + 1) * 64], W["lngN"][:, lq:lq + 128],
                                     d["y2"][0:1, :], start=True, stop=True, skip_group_check=True)
                    nc.tensor.matmul(tl[:, 128 + m * 64:128 + (m + 1) * 64], W["lnT"][:, lq:lq + 128],
                                     trhs[l][:, :], start=True, stop=True, skip_group_check=True)
                nc.vector.scalar_tensor_tensor(out=d["z"][:, :], in0=d["P"][:, :], scalar=0.0,
                                               in1=tl[:, 0:128], op0=OP.add, op1=OP.mult)
                nc.vector.scalar_tensor_tensor(out=d["z2"][:, :], in0=d["z"][:, :], scalar=0.0,
                                               in1=tl[:, 128:256], op0=OP.add, op1=OP.add)
                nc.scalar.activation(h_next[:, :], d["z2"][:, :], AF.Tanh)

            r0, r1 = rings[0], rings[1]

            def a_phases(tau, xc):
                """A = layer0 tick tau: [s1, s2, s3, s4, tail_a, tail_b]."""
                hs = hz if tau == 0 else r0[(tau - 1) % 4]
                return [
                    lambda: emit_stage(0, 1, xc=xc, h_self=hs),
                    lambda: emit_stage(0, 2, h_self=hs),
                    lambda: emit_stage(0, 3, h_self=hs),
                    lambda: emit_stage(0, 4, h_self=hs),
                    lambda: emit_tail_a(0, hs),
                    lambda: emit_tail_b(0, r0[tau % 4]),
                ]

            def b_phases(tau):
                """B = layer1 tick tau-1 (emitted during slots tau/tau+1).
                For tau==1, r1[3] is pre-zeroed and serves as h1[-1]."""
                hs = r1[(tau - 2) % 4]
                x1 = r0[(tau - 1) % 4]
                return [
                    lambda: emit_stage(1, 1, x1=x1, h_self=hs),
                    lambda: emit_stage(1, 2, h_self=hs),
                    lambda: emit_stage(1, 3, h_self=hs),
                    lambda: emit_stage(1, 4, h_self=hs),
                    lambda: emit_tail_a(1, hs),
                    lambda: emit_tail_b(1, r1[(tau - 1) % 4]),
                ]

            def emit_slot(a_ph, b_prev, b_cur):
                """Steady-state slot: A's 6 phases; B(prev slot) finishes its
                last 3 phases under A's first stages, B(cur) starts its first
                3 under A's tail."""
                order = []
                if a_ph:
                    order.append(a_ph[0])
                if b_prev:
                    order.append(b_prev[3])
                if a_ph:
                    order.append(a_ph[1])
                if b_prev:
                    order.append(b_prev[4])
                if a_ph:
                    order.append(a_ph[2])
                if b_prev:
                    order.append(b_prev[5])
                if a_ph:
                    order.append(a_ph[3])
                if b_cur:
                    order.append(b_cur[0])
                if a_ph:
                    order.append(a_ph[4])
                if b_cur:
                    order.append(b_cur[1])
                if a_ph:
                    order.append(a_ph[5])
                if b_cur:
                    order.append(b_cur[2])
                for f in order:
                    f()

            # ---- head: slots 0..1 ----------------------------------------
            xc0 = xc_pool.tile([97, BC], bf16, tag="xc")
            nc.sync.dma_start(xc0[:], xcat[0])
            emit_slot(a_phases(0, xc0), None, None)
            xc1 = xc_pool.tile([97, BC], bf16, tag="xc")
            nc.sync.dma_start(xc1[:], xcat[1])
            emit_slot(a_phases(1, xc1), None, b_phases(1))

            # ---- main loop: slots 2..505 ---------------------------------
            with tc.For_i(LOOP_LO, LOOP_HI, unroll) as iv:
                xslab = xcat[ds(iv, unroll)]
                xcu = []
                for u in range(unroll):
                    t_ = xc_pool.tile([97, BC], bf16, tag="xc")
                    nc.sync.dma_start(t_[:], xslab[u])
                    xcu.append(t_)
                for u in range(unroll):
                    tau = LOOP_LO + u  # slot = iv+u; mod-4 matches since iv%8==2
                    emit_slot(a_phases(tau, xcu[u]), b_phases(tau - 1), b_phases(tau))

            # ---- tail: slots 506..513 ------------------------------------
            for tau in range(LOOP_HI, T_run + 2):
                do_a = tau < T_run
                if do_a:
                    xct = xc_pool.tile([97, BC], bf16, tag="xc")
                    nc.sync.dma_start(xct[:], xcat[tau])
                    ap = a_phases(tau, xct)
                else:
                    ap = None
                bp_prev = b_phases(tau - 1) if tau - 1 <= T_run else None
                bp_cur = b_phases(tau) if tau <= T_run else None
                emit_slot(ap, bp_prev, bp_cur)

            nc.sync.dma_start(hout[:], rings[1][(T_run - 1) % 4][:])

    nc.compile()
    return nc


# ---------------------------------------------------------------------------
# Host-side weight prep
# ---------------------------------------------------------------------------

def _prep_weights(inputs):
    import ml_dtypes
    bf = ml_dtypes.bfloat16
    W = {k: np.asarray(v, np.float32) for k, v in inputs.items()}
    out = {}

    def pack_kt(wT):  # [256, 256] -> [128, 512] (kt, m)
        return np.concatenate([wT[0:128, :], wT[128:256, :]], axis=1)

    for l in range(L):
        fin = FS + FC if l == 0 else H
        Wg, Win, Wrec = W[f'Wg{l}'], W[f'Win{l}'], W[f'Wrec{l}']
        bg, tau = W[f'bg{l}'], W[f'tau{l}']
        itau = (1.0 / (softplus_np(tau) + 1.0)).astype(np.float32)
        WgxT = Wg[:, :fin].T
        WghT = Wg[:, fin:].T
        WrecT = Wrec.T
        WinT = Win.T
        dn = np.zeros((128, 256), np.float32)
        dnh = np.zeros((128, 256), np.float32)
        for m in range(2):
            dn[:, m * 128:(m + 1) * 128] = np.diag(-itau[m * 128:(m + 1) * 128])
            dnh[:, m * 128:(m + 1) * 128] = np.diag(-0.5 * itau[m * 128:(m + 1) * 128])
        if l == 0:
            out["g_x0"] = np.concatenate([WgxT, bg[None, :]], 0).astype(bf)
            out["in_x0"] = WinT.astype(bf)
            out["g_h0"] = pack_kt(WghT).astype(bf)
            out["g_h0h"] = pack_kt(WghT * 0.5).astype(bf)
            out["rec0"] = pack_kt(WrecT).astype(bf)
            out["rec0h"] = pack_kt(WrecT * 0.5).astype(bf)
            out["dneg0"] = dn.astype(bf)
            out["dneg0h"] = dnh.astype(bf)
        else:
            out["g_x1"] = pack_kt(WgxT).astype(bf)
            out["bg1row"] = bg[None, :].astype(bf)
            out["in_x1"] = pack_kt(WinT).astype(bf)
            out["g_h1"] = pack_kt(WghT).astype(bf)
            out["g_h1h"] = pack_kt(WghT * 0.5).astype(bf)
            out["rec1"] = pack_kt(WrecT).astype(bf)
            out["rec1h"] = pack_kt(WrecT * 0.5).astype(bf)
            out["dneg1"] = dn.astype(bf)
            out["dneg1h"] = dnh.astype(bf)
    lnT = np.zeros((2, 512), np.float32)
    lngN = np.zeros((1, 512), np.float32)
    for l in range(L):
        lng, lnb = W[f'lng{l}'], W[f'lnb{l}']
        for m in range(2):
            lq = (2 * l + m) * 128
            lnT[0, lq:lq + 128] = -lng[m * 128:(m + 1) * 128]
            lnT[1, lq:lq + 128] = lnb[m * 128:(m + 1) * 128]
            lngN[0, lq:lq + 128] = lng[m * 128:(m + 1) * 128] * N_H
    out["lnT"] = lnT.astype(bf)
    out["lngN"] = lngN.astype(bf)
    out["ones_red"] = np.ones((128, 1), np.float32).astype(bf)
    out["ones2"] = np.ones((2, 64), np.float32).astype(bf)
    return out


def _prep_core_inputs(inputs, wpack, core, T_run=T):
    seq = np.asarray(inputs['seq_features'], np.float32)
    ctx = np.asarray(inputs['context_features'], np.float32)
    bsl = slice(core * BC, (core + 1) * BC)
    import ml_dtypes
    xc = np.empty((T_run, 97, BC), np.float32)
    xc[:, 0:64, :] = seq[bsl, :T_run].transpose(1, 2, 0)
    xc[:, 64:96, :] = ctx[bsl].T[None, :, :]
    xc[:, 96, :] = 1.0
    m = {"xcat": xc.astype(ml_dtypes.bfloat16)}
    m.update(wpack)
    return m


def _head(inputs, h1):
    cW1 = np.asarray(inputs['cW1'], np.float32)
    cb1 = np.asarray(inputs['cb1'], np.float32)
    cW2 = np.asarray(inputs['cW2'], np.float32)
    cb2 = np.asarray(inputs['cb2'], np.float32)
    hid = np.maximum(h1 @ cW1.T + cb1, 0)
    return (hid @ cW2.T + cb2).squeeze(-1)


_CACHE = {}


def kernel(**inputs):
    if "nc" not in _CACHE:
        _CACHE["nc"] = build_module(T, UNROLL)
    nc = _CACHE["nc"]
    from concourse.bass_utils import run_bass_kernel_spmd
    wpack = _prep_weights(inputs)
    in_maps = [_prep_core_inputs(inputs, wpack, c) for c in range(NCORES)]
    do_trace = os.environ.get("BASS_KERNEL_TRACE") == "1"
    r = run_bass_kernel_spmd(nc, in_maps, list(range(NCORES)), trace=do_trace)
    res = r.results
    if do_trace:
        _CACHE["exec_ns"] = r.exec_time_ns
        if r.instructions_and_trace is not None:
            _CACHE["trace_path"] = r.instructions_and_trace[1]
    h1 = np.empty((B, H), np.float32)
    for c in range(NCORES):
        ht = np.asarray(res[c]["hout"], np.float32)  # [128, (m,b)]
        bsl = slice(c * BC, (c + 1) * BC)
        for m in range(2):
            h1[bsl, m * 128:(m + 1) * 128] = ht[:, m * 64:(m + 1) * 64].T
    return _head(inputs, h1).astype(np.float32)


if __name__ == "__main__":
    pass


# revision 15
# speedup vs baseline: 1.0811x; 1.0811x over previous
"""Trainium2 Bass kernel for nn_LiquidNeuralNetwork (v2).

Strategy: data-parallel over batch (8 cores x 64). Per core, a fully on-chip
recurrence over T=512 steps with the two LTC layers run as TWO INDEPENDENT
INSTRUCTION STREAMS (layer0 at tick t, layer1 at tick t-1), interleaved at
RK4-stage granularity so the tensor-engine matmuls of one stream hide the
scalar/vector latency of the other.

Per-layer tiles are [128 part = h%128, free = m*64 + b] (m = h//128 output
half, b = batch-in-core). The gate sigmoid(tanh(u)) is replaced by the fitted
a*tanh(b*u)+0.5 (max abs err 6.7e-4) so each RK4 stage costs one ScalarE tanh
plus one fused DVE (t*a+0.5)*R multiply. The LayerNorm rsqrt runs as 4 custom
DVE row ops (linear-seed + reciprocal_approx_fast + 2 Newton steps in w-form).
RK4 P-assembly and dr-deltas run on the otherwise-idle GpSimd engine.
"""

import os
import sys
import numpy as np

sys.path.insert(0, "/opt/trn_rl_repo")

B, T, FS, FC, H, L = 512, 512, 64, 32, 256, 2
LN_EPS = 1e-5
NCORES = 8
BC = B // NCORES       # 64 batch per core
UNROLL = 8
N_H = 256.0

# gate fit: sigmoid(tanh(u)) ~= GATE_A * tanh(GATE_B * u) + 0.5
GATE_A, GATE_B = 0.230386, 1.072557

# rsqrt(vv) over observed vv range [1.0e5, 1.7e6] with 4x safety margin
VLO, VHI = 2.5e4, 6.8e6
_VC = float(np.sqrt(VLO * VHI))
SEED_B = 0.5 / float(np.sqrt(_VC))
SEED_A = 1.0 / (4.0 * SEED_B)
# w = SEED_B*(N*S2 + 36*N^2*eps - S1^2) + SEED_A ~= sqrt(vv)
VVW_C0 = SEED_B * N_H
VVW_C1 = SEED_A + SEED_B * 36.0 * N_H * N_H * LN_EPS
VVW_C2 = SEED_B
# Newton in w-form: y' = (1.5 - w*y^2*(0.5/b) + (0.5a/b)*y^2) * y
NRW_C0 = 1.5
NRW_C1 = 0.5 / SEED_B
NRW_C2 = 0.5 * SEED_A / SEED_B

USE_GPSIMD = os.environ.get("LNN_NO_GPSIMD") != "1"


def softplus_np(x):
    return np.log1p(np.exp(-np.abs(x))) + np.maximum(x, 0)


# ---------------------------------------------------------------------------
# Custom DVE ops
# ---------------------------------------------------------------------------

_OPS_CACHE = {}


def _get_custom_ops():
    if _OPS_CACHE:
        return _OPS_CACHE
    from concourse.dve_spec import Spec, Src0, Src1, C0, C1, C2, lower, sq
    from concourse.dve_spec import _has_src1
    from concourse.dve_uop import DveOpSpec
    from concourse import dve_ops

    _m = sq(Src1)
    defs = {
        # out = (in0*s0 + s1) * in1   -- gate affine folded into the R-multiply
        "GATE_MUL_LNN": (
            (Src0 * C0 + C1) * Src1,
            lambda in0, in1, s0, s1, imm2: (
                (in0.astype(np.float32) * s0 + s1) * in1
            ).astype(np.float32),
        ),
        # out = (in0*s0 + s1) - in1^2 * imm2   -- w = b*(N*S2 + c - S1^2) + a
        "VV_W_LNN": (
            (Src0 * C0 + C1) - sq(Src1) * C2,
            lambda in0, in1, s0, s1, imm2: (
                (in0.astype(np.float32) * s0 + s1)
                - np.square(in1.astype(np.float32)) * imm2
            ).astype(np.float32),
        ),
        # out = (s0 - in0*in1^2*s1 + imm2*in1^2) * in1  -- Newton step, w-form
        "NR_W_LNN": (
            (C0 - Src0 * _m * C1 + C2 * _m) * Src1,
            lambda in0, in1, s0, s1, imm2: (
                (
                    s0
                    - in0.astype(np.float32) * np.square(in1.astype(np.float32)) * s1
                    + imm2 * np.square(in1.astype(np.float32))
                )
                * in1
            ).astype(np.float32),
        ),
    }
    for name, (body, ref) in defs.items():
        if name in dve_ops._SUB_OPCODE_FOR_NAME:
            _OPS_CACHE[name] = next(o for o in dve_ops.OPS if o.name == name)
            continue
        spec = Spec(body=body, reference=ref)
        opcode = dve_ops._CUSTOM_DVE_ROW_BASE + len(dve_ops.OPS)
        shas = {}
        for ver in ("v3", "v4"):
            shas[ver] = DveOpSpec(
                name=name,
                opcode=opcode,
                uops=lower(spec, ver=ver),
                rd1_en=_has_src1(spec),
            ).sha(ver)
        op = dve_ops.DveOp(name, spec, subdim=False, uops_sha=shas)
        dve_ops.OPS.append(op)
        dve_ops._SUB_OPCODE_FOR_NAME[name] = opcode
        dve_ops.CUSTOM_DVE_SPECS[name] = spec
        _OPS_CACHE[name] = op
    return _OPS_CACHE


# ---------------------------------------------------------------------------
# Bass module builder
# ---------------------------------------------------------------------------

def build_module(T_run=T, unroll=UNROLL):
    import concourse.bass as bass
    import concourse.mybir as mybir
    from concourse import tile, bacc
    from concourse.bass import ds

    ops = _get_custom_ops()
    GATE_MUL = ops["GATE_MUL_LNN"]
    VV_W = ops["VV_W_LNN"]
    NR_W = ops["NR_W_LNN"]

    f32 = mybir.dt.float32
    bf16 = mybir.dt.bfloat16
    AF = mybir.ActivationFunctionType
    OP = mybir.AluOpType

    nc = bacc.Bacc(None, target_bir_lowering=False)

    xcat = nc.declare_dram_parameter("xcat", [T_run, 97, BC], bf16, isOutput=False)
    wdecl = {}

    def wparam(name, shape):
        wdecl[name] = nc.declare_dram_parameter(name, shape, bf16, isOutput=False)
        return wdecl[name]

    wparam("g_x0", [97, 256])
    wparam("in_x0", [96, 256])
    wparam("g_h0", [128, 512])
    wparam("g_h0h", [128, 512])
    wparam("rec0", [128, 512])
    wparam("rec0h", [128, 512])
    wparam("dneg0", [128, 256])
    wparam("dneg0h", [128, 256])
    wparam("g_x1", [128, 512])
    wparam("bg1row", [1, 256])
    wparam("in_x1", [128, 512])
    wparam("g_h1", [128, 512])
    wparam("g_h1h", [128, 512])
    wparam("rec1", [128, 512])
    wparam("rec1h", [128, 512])
    wparam("dneg1", [128, 256])
    wparam("dneg1h", [128, 256])
    wparam("lnT", [2, 512])      # rows: [-lng ; lnb], col blocks (2l+m)*128
    wparam("lngN", [1, 512])     # lng * N
    wparam("ones_red", [128, 1])
    wparam("ones2", [2, 64])
    hout = nc.declare_dram_parameter("hout", [128, 128], bf16, isOutput=True)

    # slots 2..505 in the hw loop; 0..1 head; 506..513 tail
    LOOP_LO, LOOP_HI = 2, 506
    assert (LOOP_HI - LOOP_LO) % unroll == 0

    with tile.TileContext(nc) as tc:
        from contextlib import ExitStack
        with ExitStack() as ctx:
            singles = ctx.enter_context(tc.tile_pool(name="singles", bufs=1))
            xc_pool = ctx.enter_context(tc.tile_pool(name="xc", bufs=12))
            # PSUM: one bank each for G/R/V per layer; bc+stats share a bank.
            gps = [ctx.enter_context(tc.tile_pool(name=f"g{l}ps", bufs=1, space="PSUM")) for l in range(L)]
            rps = [ctx.enter_context(tc.tile_pool(name=f"r{l}ps", bufs=1, space="PSUM")) for l in range(L)]
            vps = [ctx.enter_context(tc.tile_pool(name=f"v{l}ps", bufs=1, space="PSUM")) for l in range(L)]
            tailps = [ctx.enter_context(tc.tile_pool(name=f"tail{l}ps", bufs=1, space="PSUM")) for l in range(L)]

            # ---- resident weights ---------------------------------------
            W = {}
            for name, dram in wdecl.items():
                t_ = singles.tile(list(dram.shape), bf16, name=name, tag=name)
                nc.sync.dma_start(t_[:], dram[:])
                W[name] = t_

            rings = [[singles.tile([128, 128], bf16, name=f"ring{l}_{i}", tag=f"ring{l}_{i}")
                      for i in range(4)] for l in range(L)]
            hz = singles.tile([128, 128], bf16)
            nc.vector.memset(hz[:], 0.0)
            nc.vector.memset(rings[1][3][:], 0.0)   # h1[-1] = 0
            onesr = singles.tile([1, BC], bf16)
            nc.vector.memset(onesr[:], 1.0)
            trhs = [singles.tile([2, BC], bf16, name=f"trhs{l}", tag=f"trhs{l}") for l in range(L)]
            for l in range(L):
                nc.sync.dma_start(trhs[l][:, :], wdecl["ones2"][:, :])

            # static per-stream work tiles (fixed names -> the software
            # pipeline can reference them across the hw-loop back edge)
            def mk(l, nm, dt):
                return singles.tile([128, 128], dt, name=f"{nm}{l}", tag=f"{nm}{l}")

            ST = []
            for l in range(L):
                d = dict(
                    tg=mk(l, "tg", f32), mm=mk(l, "mm", f32),
                    k1=mk(l, "k1", bf16), k2=mk(l, "k2", bf16),
                    k3=mk(l, "k3", bf16),
                    dr2=mk(l, "dr2", bf16), dr3=mk(l, "dr3", bf16),
                    c2=mk(l, "c2", f32), c3=mk(l, "c3", f32),
                    cP=mk(l, "cP", f32),
                    e1=mk(l, "e1", f32), e2=mk(l, "e2", f32),
                    e3=mk(l, "e3", f32), e4=mk(l, "e4", f32),
                    h2=mk(l, "h2", f32), h4=mk(l, "h4", f32),
                    h6=mk(l, "h6", f32), q4=mk(l, "q4", f32),
                    P=mk(l, "P", bf16), P2=mk(l, "P2", bf16),
                    z=mk(l, "z", f32), z2=mk(l, "z2", f32),
                )
                for nm in ("sx", "w", "y0", "y1"):
                    d[nm] = singles.tile([1, BC], f32, name=nm + str(l), tag=nm + str(l))
                d["y2"] = singles.tile([1, BC], bf16, name=f"y2{l}", tag=f"y2{l}")
                d["G"] = gps[l].tile([128, 128], f32, name=f"G{l}", tag="G")
                d["R"] = rps[l].tile([128, 128], f32, name=f"R{l}", tag="R")
                d["V"] = vps[l].tile([128, 128], f32, name=f"V{l}", tag="V")
                d["tail"] = tailps[l].tile([128, 512], f32, name=f"tail{l}", tag="tail")
                ST.append(d)

            def wt(name, kt, m):
                return W[name][:, kt * 256 + m * 128: kt * 256 + (m + 1) * 128]

            def dneg(l, half, m):
                nm = f"dneg{l}" + ("h" if half else "")
                return W[nm][:, m * 128:(m + 1) * 128]

            eng2 = nc.gpsimd if USE_GPSIMD else nc.vector

            def g_sub(out, a, b):  # out = a - b  (SBUF-only operands)
                if USE_GPSIMD:
                    eng2.tensor_sub(out, a, b)
                else:
                    nc.vector.scalar_tensor_tensor(out=out, in0=b, scalar=-1.0,
                                                   in1=a, op0=OP.mult, op1=OP.add)

            def g_add(out, a, b):
                if USE_GPSIMD:
                    eng2.tensor_add(out, a, b)
                else:
                    nc.vector.scalar_tensor_tensor(out=out, in0=a, scalar=0.0,
                                                   in1=b, op0=OP.add, op1=OP.add)

            def g_mul(out, a, b):
                if USE_GPSIMD:
                    eng2.tensor_mul(out, a, b)
                else:
                    nc.vector.scalar_tensor_tensor(out=out, in0=a, scalar=0.0,
                                                   in1=b, op0=OP.add, op1=OP.mult)

            # per-stream bookkeeping (mm-group first flags survive one tick)
            first_flags = [{}, {}]

            def emit_stage(l, s, xc=None, x1=None, h_self=None):
                """RK4 stage s for layer l: G MMs, gate tanh, R/V MMs, fused
                gate multiply, k, and the next stage's dr."""
                d = ST[l]
                first = first_flags[l]

                def Gc(m):
                    return d["G"][:, m * 64:(m + 1) * 64]

                def Rc(m):
                    return d["R"][:, m * 64:(m + 1) * 64]

                def Vc(m):
                    return d["V"][:, m * 64:(m + 1) * 64]

                def bmm(bank, out_ap, lhsT, rhs, last=False):
                    st_ = bank not in first
                    first[bank] = True
                    nc.tensor.matmul(out_ap, lhsT, rhs, start=st_, stop=last,
                                     skip_group_check=True)

                dr = {1: None, 2: d["k1"], 3: d["dr2"], 4: d["dr3"]}[s]
                if s == 1:
                    first.clear()
                    g_add(d["h2"][:, :], h_self[:, :], h_self[:, :])
                    g_add(d["h4"][:, :], d["h2"][:, :], d["h2"][:, :])
                    g_add(d["h6"][:, :], d["h4"][:, :], d["h2"][:, :])
                    if l == 0:
                        for m in range(2):
                            bmm("G", Gc(m), W["g_x0"][:, m * 128:(m + 1) * 128], xc[:, :])
                            for kt in range(2):
                                bmm("G", Gc(m), wt("g_h0", kt, m), h_self[:, kt * 64:(kt + 1) * 64])
                    else:
                        for m in range(2):
                            for kt in range(2):
                                bmm("G", Gc(m), wt("g_x1", kt, m), x1[:, kt * 64:(kt + 1) * 64])
                            bmm("G", Gc(m), W["bg1row"][:, m * 128:(m + 1) * 128], onesr[:, :])
                            for kt in range(2):
                                bmm("G", Gc(m), wt("g_h1", kt, m), h_self[:, kt * 64:(kt + 1) * 64])
                    nc.scalar.activation(d["tg"][:, :], d["G"][:, :], AF.Tanh, scale=GATE_B)
                    for m in range(2):
                        for kt in range(2):
                            bmm("R", Rc(m), wt(f"rec{l}", kt, m), h_self[:, kt * 64:(kt + 1) * 64])
                    if l == 0:
                        for m in range(2):
                            bmm("V", Vc(m), W["in_x0"][:, m * 128:(m + 1) * 128], xc[0:96, :])
                            bmm("V", Vc(m), dneg(0, False, m), h_self[:, m * 64:(m + 1) * 64])
                    else:
                        for m in range(2):
                            for kt in range(2):
                                bmm("V", Vc(m), wt("in_x1", kt, m), x1[:, kt * 64:(kt + 1) * 64])
                            bmm("V", Vc(m), dneg(1, False, m), h_self[:, m * 64:(m + 1) * 64])
                else:
                    half = s in (2, 3)
                    sfx = "h" if half else ""
                    last = s == 4
                    for m in range(2):
                        for kt in range(2):
                            bmm("G", Gc(m), wt(f"g_h{l}{sfx}", kt, m),
                                dr[:, kt * 64:(kt + 1) * 64], last=last and m == 1 and kt == 1)
                    nc.scalar.activation(d["tg"][:, :], d["G"][:, :], AF.Tanh, scale=GATE_B)
                    for m in range(2):
                        for kt in range(2):
                            bmm("R", Rc(m), wt(f"rec{l}{sfx}", kt, m),
                                dr[:, kt * 64:(kt + 1) * 64], last=last and m == 1 and kt == 1)
                    for m in range(2):
                        bmm("V", Vc(m), dneg(l, half, m),
                            dr[:, m * 64:(m + 1) * 64], last=last and m == 1)
                if s == 2:
                    # c2 = V - k1 (off critical path, runs under the tanh)
                    nc.vector.scalar_tensor_tensor(out=d["c2"][:, :], in0=d["k1"][:, :],
                                                   scalar=-1.0, in1=d["V"][:, :],
                                                   op0=OP.mult, op1=OP.add)
                    g_add(d["e1"][:, :], d["k1"][:, :], d["h6"][:, :])   # q1 = k1 + 6h
                elif s == 3:
                    nc.vector.scalar_tensor_tensor(out=d["c3"][:, :], in0=d["k2"][:, :],
                                                   scalar=-0.5, in1=d["V"][:, :],
                                                   op0=OP.mult, op1=OP.add)
                    g_add(d["e2"][:, :], d["k2"][:, :], d["k2"][:, :])   # q2 = 2k2
                    g_add(d["e3"][:, :], d["e1"][:, :], d["e2"][:, :])   # q3 = q1 + q2
                elif s == 4:
                    g_add(d["q4"][:, :], d["k3"][:, :], d["k3"][:, :])   # 2k3
                    g_add(d["e4"][:, :], d["e3"][:, :], d["q4"][:, :])
                    nc.vector.scalar_tensor_tensor(out=d["cP"][:, :], in0=d["e4"][:, :],
                                                   scalar=0.0, in1=d["V"][:, :],
                                                   op0=OP.add, op1=OP.add)
                nc.vector._custom_dve(GATE_MUL, out=d["mm"][:, :], in0=d["tg"][:, :],
                                      in1=d["R"][:, :], s0=GATE_A, s1=0.5)
                if s == 1:
                    nc.vector.scalar_tensor_tensor(out=d["k1"][:, :], in0=d["mm"][:, :],
                                                   scalar=0.0, in1=d["V"][:, :],
                                                   op0=OP.add, op1=OP.add)
                elif s == 2:
                    nc.vector.scalar_tensor_tensor(out=d["dr2"][:, :], in0=d["mm"][:, :],
                                                   scalar=0.0, in1=d["c2"][:, :],
                                                   op0=OP.add, op1=OP.add)
                    nc.vector.scalar_tensor_tensor(out=d["k2"][:, :], in0=d["mm"][:, :],
                                                   scalar=0.0, in1=d["V"][:, :],
                                                   op0=OP.add, op1=OP.add)
                elif s == 3:
                    nc.vector.scalar_tensor_tensor(out=d["dr3"][:, :], in0=d["mm"][:, :],
                                                   scalar=0.0, in1=d["c3"][:, :],
                                                   op0=OP.add, op1=OP.add)
                    nc.vector.scalar_tensor_tensor(out=d["k3"][:, :], in0=d["mm"][:, :],
                                                   scalar=0.0, in1=d["V"][:, :],
                                                   op0=OP.add, op1=OP.add)
                else:
                    nc.vector.scalar_tensor_tensor(out=d["P"][:, :], in0=d["mm"][:, :],
                                                   scalar=0.0, in1=d["cP"][:, :],
                                                   op0=OP.add, op1=OP.add)

            def emit_tail_a(l, h_self):
                d = ST[l]
                tl = d["tail"]
                nc.tensor.matmul(tl[0:1, 256:320], W["ones_red"][:, :], d["P"][:, 0:64],
                                 start=True, stop=False, skip_group_check=True)
                nc.tensor.matmul(tl[0:1, 256:320], W["ones_red"][:, :], d["P"][:, 64:128],
                                 start=False, stop=True, skip_group_check=True)
                nc.scalar.activation(d["sx"][:, :], tl[0:1, 256:320], AF.Copy)
                nc.scalar.activation(d["P2"][:, :], d["P"][:, :], AF.Square)
                nc.tensor.matmul(tl[0:1, 320:384], W["ones_red"][:, :], d["P2"][:, 0:64],
                                 start=True, stop=False, skip_group_check=True)
                nc.tensor.matmul(tl[0:1, 320:384], W["ones_red"][:, :], d["P2"][:, 64:128],
                                 start=False, stop=True, skip_group_check=True)

            def emit_tail_b(l, h_next):
                d = ST[l]
                tl = d["tail"]
                s2_ap = tl[0:1, 320:384]
                nc.vector._custom_dve(VV_W, out=d["w"][:, :], in0=s2_ap,
                                      in1=d["sx"][:, :], s0=VVW_C0, s1=VVW_C1, imm2=VVW_C2)
                nc.vector.reciprocal_approx_fast(out=d["y0"][:, :], in_=d["w"][:, :])
                nc.vector._custom_dve(NR_W, out=d["y1"][:, :], in0=d["w"][:, :], in1=d["y0"][:, :],
                                      s0=NRW_C0, s1=NRW_C1, imm2=NRW_C2)
                nc.vector._custom_dve(NR_W, out=d["y2"][:, :], in0=d["w"][:, :], in1=d["y1"][:, :],
                                      s0=NRW_C0, s1=NRW_C1, imm2=NRW_C2)
                g_mul(trhs[l][0:1, :], d["sx"][:, :], d["y2"][:, :])
                for m in range(2):
                    lq = (2 * l + m) * 128
                    nc.tensor.matmul(tl[:, m * 64:(m + 1) * 64], W["lngN"][:, lq:lq + 128],
                                     d["y2"][0:1, :], start=True, stop=True, skip_group_check=True)
                    nc.tensor.matmul(tl[:, 128 + m * 64:128 + (m + 1) * 64], W["lnT"][:, lq:lq + 128],
                                     trhs[l][:, :], start=True, stop=True, skip_group_check=True)
                nc.vector.scalar_tensor_tensor(out=d["z"][:, :], in0=d["P"][:, :], scalar=0.0,
                                               in1=tl[:, 0:128], op0=OP.add, op1=OP.mult)
                nc.vector.scalar_tensor_tensor(out=d["z2"][:, :], in0=d["z"][:, :], scalar=0.0,
                                               in1=tl[:, 128:256], op0=OP.add, op1=OP.add)
                nc.scalar.activation(h_next[:, :], d["z2"][:, :], AF.Tanh)

            r0, r1 = rings[0], rings[1]

            def a_phases(tau, xc):
                """A = layer0 tick tau: [s1, s2, s3, s4, tail_a, tail_b]."""
                hs = hz if tau == 0 else r0[(tau - 1) % 4]
                return [
                    lambda: emit_stage(0, 1, xc=xc, h_self=hs),
                    lambda: emit_stage(0, 2, h_self=hs),
                    lambda: emit_stage(0, 3, h_self=hs),
                    lambda: emit_stage(0, 4, h_self=hs),
                    lambda: emit_tail_a(0, hs),
                    lambda: emit_tail_b(0, r0[tau % 4]),
                ]

            def b_phases(tau):
                """B = layer1 tick tau-1 (emitted during slots tau/tau+1).
                For tau==1, r1[3] is pre-zeroed and serves as h1[-1]."""
                hs = r1[(tau - 2) % 4]
                x1 = r0[(tau - 1) % 4]
                return [
                    lambda: emit_stage(1, 1, x1=x1, h_self=hs),
                    lambda: emit_stage(1, 2, h_self=hs),
                    lambda: emit_stage(1, 3, h_self=hs),
                    lambda: emit_stage(1, 4, h_self=hs),
                    lambda: emit_tail_a(1, hs),
                    lambda: emit_tail_b(1, r1[(tau - 1) % 4]),
                ]

            def emit_slot(a_ph, b_prev, b_cur):
                """Steady-state slot: A's 6 phases; B(prev slot) finishes its
                last 3 phases under A's first stages, B(cur) starts its first
                3 under A's tail."""
                order = []
                if a_ph:
                    order.append(a_ph[0])
                if b_prev:
                    order.append(b_prev[3])
                if a_ph:
                    order.append(a_ph[1])
                if b_prev:
                    order.append(b_prev[4])
                if a_ph:
                    order.append(a_ph[2])
                if b_prev:
                    order.append(b_prev[5])
                if a_ph:
                    order.append(a_ph[3])
                if b_cur:
                    order.append(b_cur[0])
                if a_ph:
                    order.append(a_ph[4])
                if b_cur:
                    order.append(b_cur[1])
                if a_ph:
                    order.append(a_ph[5])
                if b_cur:
                    order.append(b_cur[2])
                for f in order:
                    f()

            # ---- head: slots 0..1 ----------------------------------------
            xc0 = xc_pool.tile([97, BC], bf16, tag="xc")
            nc.sync.dma_start(xc0[:], xcat[0])
            emit_slot(a_phases(0, xc0), None, None)
            xc1 = xc_pool.tile([97, BC], bf16, tag="xc")
            nc.sync.dma_start(xc1[:], xcat[1])
            emit_slot(a_phases(1, xc1), None, b_phases(1))

            # ---- main loop: slots 2..505 ---------------------------------
            with tc.For_i(LOOP_LO, LOOP_HI, unroll) as iv:
                xslab = xcat[ds(iv, unroll)]
                xcu = []
                for u in range(unroll):
                    t_ = xc_pool.tile([97, BC], bf16, tag="xc")
                    nc.sync.dma_start(t_[:], xslab[u])
                    xcu.append(t_)
                for u in range(unroll):
                    tau = LOOP_LO + u  # slot = iv+u; mod-4 matches since iv%8==2
                    emit_slot(a_phases(tau, xcu[u]), b_phases(tau - 1), b_phases(tau))

            # ---- tail: slots 506..513 ------------------------------------
            for tau in range(LOOP_HI, T_run + 2):
                do_a = tau < T_run
                if do_a:
                    xct = xc_pool.tile([97, BC], bf16, tag="xc")
                    nc.sync.dma_start(xct[:], xcat[tau])
                    ap = a_phases(tau, xct)
                else:
                    ap = None
                bp_prev = b_phases(tau - 1) if tau - 1 <= T_run else None
                bp_cur = b_phases(tau) if tau <= T_run else None
                emit_slot(ap, bp_prev, bp_cur)

            nc.sync.dma_start(hout[:], rings[1][(T_run - 1) % 4][:])

    nc.compile()
    return nc


# ---------------------------------------------------------------------------
# Host-side weight prep
# ---------------------------------------------------------------------------

def _prep_weights(inputs):
    import ml_dtypes
    bf = ml_dtypes.bfloat16
    W = {k: np.asarray(v, np.float32) for k, v in inputs.items()}
    out = {}

    def pack_kt(wT):  # [256, 256] -> [128, 512] (kt, m)
        return np.concatenate([wT[0:128, :], wT[128:256, :]], axis=1)

    for l in range(L):
        fin = FS + FC if l == 0 else H
        Wg, Win, Wrec = W[f'Wg{l}'], W[f'Win{l}'], W[f'Wrec{l}']
        bg, tau = W[f'bg{l}'], W[f'tau{l}']
        itau = (1.0 / (softplus_np(tau) + 1.0)).astype(np.float32)
        WgxT = Wg[:, :fin].T
        WghT = Wg[:, fin:].T
        WrecT = Wrec.T
        WinT = Win.T
        dn = np.zeros((128, 256), np.float32)
        dnh = np.zeros((128, 256), np.float32)
        for m in range(2):
            dn[:, m * 128:(m + 1) * 128] = np.diag(-itau[m * 128:(m + 1) * 128])
            dnh[:, m * 128:(m + 1) * 128] = np.diag(-0.5 * itau[m * 128:(m + 1) * 128])
        if l == 0:
            out["g_x0"] = np.concatenate([WgxT, bg[None, :]], 0).astype(bf)
            out["in_x0"] = WinT.astype(bf)
            out["g_h0"] = pack_kt(WghT).astype(bf)
            out["g_h0h"] = pack_kt(WghT * 0.5).astype(bf)
            out["rec0"] = pack_kt(WrecT).astype(bf)
            out["rec0h"] = pack_kt(WrecT * 0.5).astype(bf)
            out["dneg0"] = dn.astype(bf)
            out["dneg0h"] = dnh.astype(bf)
        else:
            out["g_x1"] = pack_kt(WgxT).astype(bf)
            out["bg1row"] = bg[None, :].astype(bf)
            out["in_x1"] = pack_kt(WinT).astype(bf)
            out["g_h1"] = pack_kt(WghT).astype(bf)
            out["g_h1h"] = pack_kt(WghT * 0.5).astype(bf)
            out["rec1"] = pack_kt(WrecT).astype(bf)
            out["rec1h"] = pack_kt(WrecT * 0.5).astype(bf)
            out["dneg1"] = dn.astype(bf)
            out["dneg1h"] = dnh.astype(bf)
    lnT = np.zeros((2, 512), np.float32)
    lngN = np.zeros((1, 512), np.float32)
    for l in range(L):
        lng, lnb = W[f'lng{l}'], W[f'lnb{l}']
        for m in range(2):
            lq = (2 * l + m) * 128
            lnT[0, lq:lq + 128] = -lng[m * 128:(m + 1) * 128]
            lnT[1, lq:lq + 128] = lnb[m * 128:(m + 1) * 128]
            lngN[0, lq:lq + 128] = lng[m * 128:(m + 1) * 128] * N_H
    out["lnT"] = lnT.astype(bf)
    out["lngN"] = lngN.astype(bf)
    out["ones_red"] = np.ones((128, 1), np.float32).astype(bf)
    out["ones2"] = np.ones((2, 64), np.float32).astype(bf)
    return out


def _prep_core_inputs(inputs, wpack, core, T_run=T):
    seq = np.asarray(inputs['seq_features'], np.float32)
    ctx = np.asarray(inputs['context_features'], np.float32)
    bsl = slice(core * BC, (core + 1) * BC)
    import ml_dtypes
    xc = np.empty((T_run, 97, BC), np.float32)
    xc[:, 0:64, :] = seq[bsl, :T_run].transpose(1, 2, 0)
    xc[:, 64:96, :] = ctx[bsl].T[None, :, :]
    xc[:, 96, :] = 1.0
    m = {"xcat": xc.astype(ml_dtypes.bfloat16)}
    m.update(wpack)
    return m


def _head(inputs, h1):
    cW1 = np.asarray(inputs['cW1'], np.float32)
    cb1 = np.asarray(inputs['cb1'], np.float32)
    cW2 = np.asarray(inputs['cW2'], np.float32)
    cb2 = np.asarray(inputs['cb2'], np.float32)
    hid = np.maximum(h1 @ cW1.T + cb1, 0)
    return (hid @ cW2.T + cb2).squeeze(-1)


_CACHE = {}


def kernel(**inputs):
    if "nc" not in _CACHE:
        _CACHE["nc"] = build_module(T, UNROLL)
    nc = _CACHE["nc"]
    from concourse.bass_utils import run_bass_kernel_spmd
    wpack = _prep_weights(inputs)
    in_maps = [_prep_core_inputs(inputs, wpack, c) for c in range(NCORES)]
    do_trace = os.environ.get("BASS_KERNEL_TRACE") == "1"
    r = run_bass_kernel_spmd(nc, in_maps, list(range(NCORES)), trace=do_trace)
    res = r.results
    if do_trace:
        _CACHE["exec_ns"] = r.exec_time_ns
        if r.instructions_and_trace is not None:
            _CACHE["trace_path"] = r.instructions_and_trace[1]
    h1 = np.empty((B, H), np.float32)
    for c in range(NCORES):
        ht = np.asarray(res[c]["hout"], np.float32)  # [128, (m,b)]
        bsl = slice(c * BC, (c + 1) * BC)
        for m in range(2):
            h1[bsl, m * 128:(m + 1) * 128] = ht[:, m * 64:(m + 1) * 64].T
    return _head(inputs, h1).astype(np.float32)


if __name__ == "__main__":
    pass


# revision 17
# speedup vs baseline: 1.2106x; 1.1198x over previous
"""Trainium2 Bass kernel for nn_LiquidNeuralNetwork (v2).

Strategy: data-parallel over batch (8 cores x 64). Per core, a fully on-chip
recurrence over T=512 steps with the two LTC layers run as TWO INDEPENDENT
INSTRUCTION STREAMS (layer0 at tick t, layer1 at tick t-1), interleaved at
RK4-stage granularity so the tensor-engine matmuls of one stream hide the
scalar/vector latency of the other.

Per-layer tiles are [128 part = h%128, free = m*64 + b] (m = h//128 output
half, b = batch-in-core). The gate sigmoid(tanh(u)) is replaced by the fitted
a*tanh(b*u)+0.5 (max abs err 6.7e-4) so each RK4 stage costs one ScalarE tanh
plus one fused DVE (t*a+0.5)*R multiply. The LayerNorm rsqrt runs as 4 custom
DVE row ops (linear-seed + reciprocal_approx_fast + 2 Newton steps in w-form).
RK4 P-assembly and dr-deltas run on the otherwise-idle GpSimd engine.
"""

import os
import sys
import numpy as np

sys.path.insert(0, "/opt/trn_rl_repo")

B, T, FS, FC, H, L = 512, 512, 64, 32, 256, 2
LN_EPS = 1e-5
NCORES = 8
BC = B // NCORES       # 64 batch per core
UNROLL = 8
N_H = 256.0

# gate fit: sigmoid(tanh(u)) ~= GATE_A * tanh(GATE_B * u) + 0.5
GATE_A, GATE_B = 0.230386, 1.072557

# rsqrt(vv) over observed vv range [1.0e5, 1.7e6] with 4x safety margin
VLO, VHI = 2.5e4, 6.8e6
_VC = float(np.sqrt(VLO * VHI))
SEED_B = 0.5 / float(np.sqrt(_VC))
SEED_A = 1.0 / (4.0 * SEED_B)
# w = SEED_B*(N*S2 + 36*N^2*eps - S1^2) + SEED_A ~= sqrt(vv)
VVW_C0 = SEED_B * N_H
VVW_C1 = SEED_A + SEED_B * 36.0 * N_H * N_H * LN_EPS
VVW_C2 = SEED_B
# Newton in w-form: y' = (1.5 - w*y^2*(0.5/b) + (0.5a/b)*y^2) * y
NRW_C0 = 1.5
NRW_C1 = 0.5 / SEED_B
NRW_C2 = 0.5 * SEED_A / SEED_B

USE_GPSIMD = os.environ.get("LNN_NO_GPSIMD") != "1"


def softplus_np(x):
    return np.log1p(np.exp(-np.abs(x))) + np.maximum(x, 0)


# ---------------------------------------------------------------------------
# Custom DVE ops
# ---------------------------------------------------------------------------

_OPS_CACHE = {}


def _get_custom_ops():
    if _OPS_CACHE:
        return _OPS_CACHE
    from concourse.dve_spec import Spec, Src0, Src1, C0, C1, C2, lower, sq
    from concourse.dve_spec import _has_src1
    from concourse.dve_uop import DveOpSpec
    from concourse import dve_ops

    _m = sq(Src1)
    defs = {
        # out = (in0*s0 + s1) * in1   -- gate affine folded into the R-multiply
        "GATE_MUL_LNN": (
            (Src0 * C0 + C1) * Src1,
            lambda in0, in1, s0, s1, imm2: (
                (in0.astype(np.float32) * s0 + s1) * in1
            ).astype(np.float32),
        ),
        # out = (in0*s0 + s1) - in1^2 * imm2   -- w = b*(N*S2 + c - S1^2) + a
        "VV_W_LNN": (
            (Src0 * C0 + C1) - sq(Src1) * C2,
            lambda in0, in1, s0, s1, imm2: (
                (in0.astype(np.float32) * s0 + s1)
                - np.square(in1.astype(np.float32)) * imm2
            ).astype(np.float32),
        ),
        # out = (s0 - in0*in1^2*s1 + imm2*in1^2) * in1  -- Newton step, w-form
        "NR_W_LNN": (
            (C0 - Src0 * _m * C1 + C2 * _m) * Src1,
            lambda in0, in1, s0, s1, imm2: (
                (
                    s0
                    - in0.astype(np.float32) * np.square(in1.astype(np.float32)) * s1
                    + imm2 * np.square(in1.astype(np.float32))
                )
                * in1
            ).astype(np.float32),
        ),
    }
    for name, (body, ref) in defs.items():
        if name in dve_ops._SUB_OPCODE_FOR_NAME:
            _OPS_CACHE[name] = next(o for o in dve_ops.OPS if o.name == name)
            continue
        spec = Spec(body=body, reference=ref)
        opcode = dve_ops._CUSTOM_DVE_ROW_BASE + len(dve_ops.OPS)
        shas = {}
        for ver in ("v3", "v4"):
            shas[ver] = DveOpSpec(
                name=name,
                opcode=opcode,
                uops=lower(spec, ver=ver),
                rd1_en=_has_src1(spec),
            ).sha(ver)
        op = dve_ops.DveOp(name, spec, subdim=False, uops_sha=shas)
        dve_ops.OPS.append(op)
        dve_ops._SUB_OPCODE_FOR_NAME[name] = opcode
        dve_ops.CUSTOM_DVE_SPECS[name] = spec
        _OPS_CACHE[name] = op
    return _OPS_CACHE


# ---------------------------------------------------------------------------
# Bass module builder
# ---------------------------------------------------------------------------

def build_module(T_run=T, unroll=UNROLL):
    import concourse.bass as bass
    import concourse.mybir as mybir
    from concourse import tile, bacc
    from concourse.bass import ds

    ops = _get_custom_ops()
    GATE_MUL = ops["GATE_MUL_LNN"]
    VV_W = ops["VV_W_LNN"]
    NR_W = ops["NR_W_LNN"]

    f32 = mybir.dt.float32
    bf16 = mybir.dt.bfloat16
    AF = mybir.ActivationFunctionType
    OP = mybir.AluOpType

    nc = bacc.Bacc(None, target_bir_lowering=False)

    xcat = nc.declare_dram_parameter("xcat", [T_run, 97, BC], bf16, isOutput=False)
    wdecl = {}

    def wparam(name, shape):
        wdecl[name] = nc.declare_dram_parameter(name, shape, bf16, isOutput=False)
        return wdecl[name]

    wparam("g_x0", [97, 256])
    wparam("in_x0", [96, 256])
    wparam("g_h0", [128, 512])
    wparam("g_h0h", [128, 512])
    wparam("rec0", [128, 512])
    wparam("rec0h", [128, 512])
    wparam("dneg0", [128, 256])
    wparam("dneg0h", [128, 256])
    wparam("g_x1", [128, 512])
    wparam("bg1row", [1, 256])
    wparam("in_x1", [128, 512])
    wparam("g_h1", [128, 512])
    wparam("g_h1h", [128, 512])
    wparam("rec1", [128, 512])
    wparam("rec1h", [128, 512])
    wparam("dneg1", [128, 256])
    wparam("dneg1h", [128, 256])
    wparam("lnT", [2, 512])      # rows: [-lng ; lnb], col blocks (2l+m)*128
    wparam("lngN", [1, 512])     # lng * N
    wparam("ones_red", [128, 1])
    wparam("id6", [128, 128])
    wparam("id1", [128, 128])
    wparam("id2", [128, 128])
    wparam("ones2", [2, 64])
    hout = nc.declare_dram_parameter("hout", [128, 128], bf16, isOutput=True)

    # slots 2..505 in the hw loop; 0..1 head; 506..513 tail
    LOOP_LO, LOOP_HI = 2, 506
    assert (LOOP_HI - LOOP_LO) % unroll == 0

    with tile.TileContext(nc) as tc:
        from contextlib import ExitStack
        with ExitStack() as ctx:
            singles = ctx.enter_context(tc.tile_pool(name="singles", bufs=1))
            xc_pool = ctx.enter_context(tc.tile_pool(name="xc", bufs=12))
            # PSUM: one bank each for G/R/V per layer; bc+stats share a bank.
            gps = [ctx.enter_context(tc.tile_pool(name=f"g{l}ps", bufs=1, space="PSUM")) for l in range(L)]
            rps = [ctx.enter_context(tc.tile_pool(name=f"r{l}ps", bufs=1, space="PSUM")) for l in range(L)]
            vps = [ctx.enter_context(tc.tile_pool(name=f"v{l}ps", bufs=1, space="PSUM")) for l in range(L)]
            tailps = [ctx.enter_context(tc.tile_pool(name=f"tail{l}ps", bufs=1, space="PSUM")) for l in range(L)]

            # ---- resident weights ---------------------------------------
            W = {}
            for name, dram in wdecl.items():
                t_ = singles.tile(list(dram.shape), bf16, name=name, tag=name)
                nc.sync.dma_start(t_[:], dram[:])
                W[name] = t_

            rings = [[singles.tile([128, 128], bf16, name=f"ring{l}_{i}", tag=f"ring{l}_{i}")
                      for i in range(4)] for l in range(L)]
            hz = singles.tile([128, 128], bf16)
            nc.vector.memset(hz[:], 0.0)
            nc.vector.memset(rings[1][3][:], 0.0)   # h1[-1] = 0
            onesr = singles.tile([1, BC], bf16)
            nc.vector.memset(onesr[:], 1.0)
            trhs = [singles.tile([2, BC], bf16, name=f"trhs{l}", tag=f"trhs{l}") for l in range(L)]
            for l in range(L):
                nc.sync.dma_start(trhs[l][:, :], wdecl["ones2"][:, :])

            # static per-stream work tiles (fixed names -> the software
            # pipeline can reference them across the hw-loop back edge)
            def mk(l, nm, dt):
                return singles.tile([128, 128], dt, name=f"{nm}{l}", tag=f"{nm}{l}")

            ST = []
            for l in range(L):
                d = dict(
                    tg=mk(l, "tg", f32), mm=mk(l, "mm", f32),
                    k1=mk(l, "k1", bf16), k2=mk(l, "k2", bf16),
                    k3=mk(l, "k3", bf16), k4=mk(l, "k4", bf16),
                    dr2=mk(l, "dr2", bf16), dr3=mk(l, "dr3", bf16),
                    P=mk(l, "P", bf16), P2=mk(l, "P2", bf16),
                    z=mk(l, "z", f32), z2=mk(l, "z2", f32),
                )
                for nm in ("sx", "w", "y0", "y1"):
                    d[nm] = singles.tile([1, BC], f32, name=nm + str(l), tag=nm + str(l))
                d["y2"] = singles.tile([1, BC], bf16, name=f"y2{l}", tag=f"y2{l}")
                d["G"] = gps[l].tile([128, 128], f32, name=f"G{l}", tag="G")
                d["R"] = rps[l].tile([128, 128], f32, name=f"R{l}", tag="R")
                d["V"] = vps[l].tile([128, 128], f32, name=f"V{l}", tag="V")
                d["tail"] = tailps[l].tile([128, 512], f32, name=f"tail{l}", tag="tail")
                ST.append(d)

            def wt(name, kt, m):
                return W[name][:, kt * 256 + m * 128: kt * 256 + (m + 1) * 128]

            def dneg(l, half, m):
                nm = f"dneg{l}" + ("h" if half else "")
                return W[nm][:, m * 128:(m + 1) * 128]

            eng2 = nc.gpsimd if USE_GPSIMD else nc.vector

            def g_sub(out, a, b):  # out = a - b  (SBUF-only operands)
                if USE_GPSIMD:
                    eng2.tensor_sub(out, a, b)
                else:
                    nc.vector.scalar_tensor_tensor(out=out, in0=b, scalar=-1.0,
                                                   in1=a, op0=OP.mult, op1=OP.add)

            def g_add(out, a, b):
                if USE_GPSIMD:
                    eng2.tensor_add(out, a, b)
                else:
                    nc.vector.scalar_tensor_tensor(out=out, in0=a, scalar=0.0,
                                                   in1=b, op0=OP.add, op1=OP.add)

            def g_mul(out, a, b):
                if USE_GPSIMD:
                    eng2.tensor_mul(out, a, b)
                else:
                    nc.vector.scalar_tensor_tensor(out=out, in0=a, scalar=0.0,
                                                   in1=b, op0=OP.add, op1=OP.mult)

            # per-stream bookkeeping (mm-group first flags survive one tick)
            first_flags = [{}, {}]

            def emit_stage(l, s, xc=None, x1=None, h_self=None):
                """RK4 stage s for layer l: G MMs, gate tanh, R/V MMs, fused
                gate multiply, k, and the next stage's dr."""
                d = ST[l]
                first = first_flags[l]

                def Gc(m):
                    return d["G"][:, m * 64:(m + 1) * 64]

                def Rc(m):
                    return d["R"][:, m * 64:(m + 1) * 64]

                def Vc(m):
                    return d["V"][:, m * 64:(m + 1) * 64]

                def bmm(bank, out_ap, lhsT, rhs, last=False):
                    st_ = bank not in first
                    first[bank] = True
                    nc.tensor.matmul(out_ap, lhsT, rhs, start=st_, stop=last,
                                     skip_group_check=True)

                Pp = d["tail"][:, 384:512]
                dr = {1: None, 2: d["k1"], 3: d["dr2"], 4: d["dr3"]}[s]
                if s == 1:
                    first.clear()
                    nc.tensor.matmul(Pp, W["id6"][:, :], h_self[:, :],
                                     start=True, stop=False, skip_group_check=True)
                    if l == 0:
                        for m in range(2):
                            bmm("G", Gc(m), W["g_x0"][:, m * 128:(m + 1) * 128], xc[:, :])
                            for kt in range(2):
                                bmm("G", Gc(m), wt("g_h0", kt, m), h_self[:, kt * 64:(kt + 1) * 64])
                    else:
                        for m in range(2):
                            for kt in range(2):
                                bmm("G", Gc(m), wt("g_x1", kt, m), x1[:, kt * 64:(kt + 1) * 64])
                            bmm("G", Gc(m), W["bg1row"][:, m * 128:(m + 1) * 128], onesr[:, :])
                            for kt in range(2):
                                bmm("G", Gc(m), wt("g_h1", kt, m), h_self[:, kt * 64:(kt + 1) * 64])
                    nc.scalar.activation(d["tg"][:, :], d["G"][:, :], AF.Tanh, scale=GATE_B)
                    for m in range(2):
                        for kt in range(2):
                            bmm("R", Rc(m), wt(f"rec{l}", kt, m), h_self[:, kt * 64:(kt + 1) * 64])
                    if l == 0:
                        for m in range(2):
                            bmm("V", Vc(m), W["in_x0"][:, m * 128:(m + 1) * 128], xc[0:96, :])
                            bmm("V", Vc(m), dneg(0, False, m), h_self[:, m * 64:(m + 1) * 64])
                    else:
                        for m in range(2):
                            for kt in range(2):
                                bmm("V", Vc(m), wt("in_x1", kt, m), x1[:, kt * 64:(kt + 1) * 64])
                            bmm("V", Vc(m), dneg(1, False, m), h_self[:, m * 64:(m + 1) * 64])
                else:
                    half = s in (2, 3)
                    sfx = "h" if half else ""
                    last = s == 4
                    for m in range(2):
                        for kt in range(2):
                            bmm("G", Gc(m), wt(f"g_h{l}{sfx}", kt, m),
                                dr[:, kt * 64:(kt + 1) * 64], last=last and m == 1 and kt == 1)
                    nc.scalar.activation(d["tg"][:, :], d["G"][:, :], AF.Tanh, scale=GATE_B)
                    for m in range(2):
                        for kt in range(2):
                            bmm("R", Rc(m), wt(f"rec{l}{sfx}", kt, m),
                                dr[:, kt * 64:(kt + 1) * 64], last=last and m == 1 and kt == 1)
                    for m in range(2):
                        bmm("V", Vc(m), dneg(l, half, m),
                            dr[:, m * 64:(m + 1) * 64], last=last and m == 1)
                if s >= 2:
                    # accumulate the previous stage's k into P (identity MMs)
                    kprev = d[f"k{s-1}"]
                    idw = "id1" if s == 2 else "id2"
                    nc.tensor.matmul(Pp, W[idw][:, :], kprev[:, :],
                                     start=False, stop=False, skip_group_check=True)
                nc.vector._custom_dve(GATE_MUL, out=d["mm"][:, :], in0=d["tg"][:, :],
                                      in1=d["R"][:, :], s0=GATE_A, s1=0.5)
                kk = d[f"k{s}"]
                nc.vector.scalar_tensor_tensor(out=kk[:, :], in0=d["mm"][:, :], scalar=0.0,
                                               in1=d["V"][:, :], op0=OP.add, op1=OP.add)
                if s == 2:
                    nc.vector.scalar_tensor_tensor(out=d["dr2"][:, :], in0=d["k1"][:, :],
                                                   scalar=-1.0, in1=kk[:, :],
                                                   op0=OP.mult, op1=OP.add)
                elif s == 3:
                    nc.vector.scalar_tensor_tensor(out=d["dr3"][:, :], in0=d["k2"][:, :],
                                                   scalar=-0.5, in1=kk[:, :],
                                                   op0=OP.mult, op1=OP.add)

            def emit_tail_a(l, h_self):
                d = ST[l]
                tl = d["tail"]
                Pp = tl[:, 384:512]
                nc.tensor.matmul(Pp, W["id1"][:, :], d["k4"][:, :],
                                 start=False, stop=True, skip_group_check=True)
                nc.scalar.activation(d["P"][:, :], Pp, AF.Copy)
                nc.scalar.activation(d["P2"][:, :], Pp, AF.Square)
                nc.tensor.matmul(tl[0:1, 256:320], W["ones_red"][:, :], d["P"][:, 0:64],
                                 start=True, stop=False, skip_group_check=True)
                nc.tensor.matmul(tl[0:1, 256:320], W["ones_red"][:, :], d["P"][:, 64:128],
                                 start=False, stop=True, skip_group_check=True)
                nc.scalar.activation(d["sx"][:, :], tl[0:1, 256:320], AF.Copy)
                nc.tensor.matmul(tl[0:1, 320:384], W["ones_red"][:, :], d["P2"][:, 0:64],
                                 start=True, stop=False, skip_group_check=True)
                nc.tensor.matmul(tl[0:1, 320:384], W["ones_red"][:, :], d["P2"][:, 64:128],
                                 start=False, stop=True, skip_group_check=True)

            def emit_tail_b(l, h_next):
                d = ST[l]
                tl = d["tail"]
                s2_ap = tl[0:1, 320:384]
                nc.vector._custom_dve(VV_W, out=d["w"][:, :], in0=s2_ap,
                                      in1=d["sx"][:, :], s0=VVW_C0, s1=VVW_C1, imm2=VVW_C2)
                nc.vector.reciprocal_approx_fast(out=d["y0"][:, :], in_=d["w"][:, :])
                nc.vector._custom_dve(NR_W, out=d["y1"][:, :], in0=d["w"][:, :], in1=d["y0"][:, :],
                                      s0=NRW_C0, s1=NRW_C1, imm2=NRW_C2)
                nc.vector._custom_dve(NR_W, out=d["y2"][:, :], in0=d["w"][:, :], in1=d["y1"][:, :],
                                      s0=NRW_C0, s1=NRW_C1, imm2=NRW_C2)
                g_mul(trhs[l][0:1, :], d["sx"][:, :], d["y2"][:, :])
                for m in range(2):
                    lq = (2 * l + m) * 128
                    nc.tensor.matmul(tl[:, m * 64:(m + 1) * 64], W["lngN"][:, lq:lq + 128],
                                     d["y2"][0:1, :], start=True, stop=True, skip_group_check=True)
                    nc.tensor.matmul(tl[:, 128 + m * 64:128 + (m + 1) * 64], W["lnT"][:, lq:lq + 128],
                                     trhs[l][:, :], start=True, stop=True, skip_group_check=True)
                nc.vector.scalar_tensor_tensor(out=d["z"][:, :], in0=d["P"][:, :], scalar=0.0,
                                               in1=tl[:, 0:128], op0=OP.add, op1=OP.mult)
                nc.vector.scalar_tensor_tensor(out=d["z2"][:, :], in0=d["z"][:, :], scalar=0.0,
                                               in1=tl[:, 128:256], op0=OP.add, op1=OP.add)
                nc.scalar.activation(h_next[:, :], d["z2"][:, :], AF.Tanh)

            r0, r1 = rings[0], rings[1]

            def a_phases(tau, xc):
                """A = layer0 tick tau: [s1, s2, s3, s4, tail_a, tail_b]."""
                hs = hz if tau == 0 else r0[(tau - 1) % 4]
                return [
                    lambda: emit_stage(0, 1, xc=xc, h_self=hs),
                    lambda: emit_stage(0, 2, h_self=hs),
                    lambda: emit_stage(0, 3, h_self=hs),
                    lambda: emit_stage(0, 4, h_self=hs),
                    lambda: emit_tail_a(0, hs),
                    lambda: emit_tail_b(0, r0[tau % 4]),
                ]

            def b_phases(tau):
                """B = layer1 tick tau-1 (emitted during slots tau/tau+1).
                For tau==1, r1[3] is pre-zeroed and serves as h1[-1]."""
                hs = r1[(tau - 2) % 4]
                x1 = r0[(tau - 1) % 4]
                return [
                    lambda: emit_stage(1, 1, x1=x1, h_self=hs),
                    lambda: emit_stage(1, 2, h_self=hs),
                    lambda: emit_stage(1, 3, h_self=hs),
                    lambda: emit_stage(1, 4, h_self=hs),
                    lambda: emit_tail_a(1, hs),
                    lambda: emit_tail_b(1, r1[(tau - 1) % 4]),
                ]

            def emit_slot(a_ph, b_prev, b_cur):
                """Steady-state slot: A's 6 phases; B(prev slot) finishes its
                last 3 phases under A's first stages, B(cur) starts its first
                3 under A's tail."""
                order = []
                if a_ph:
                    order.append(a_ph[0])
                if b_prev:
                    order.append(b_prev[3])
                if a_ph:
                    order.append(a_ph[1])
                if b_prev:
                    order.append(b_prev[4])
                if a_ph:
                    order.append(a_ph[2])
                if b_prev:
                    order.append(b_prev[5])
                if a_ph:
                    order.append(a_ph[3])
                if b_cur:
                    order.append(b_cur[0])
                if a_ph:
                    order.append(a_ph[4])
                if b_cur:
                    order.append(b_cur[1])
                if a_ph:
                    order.append(a_ph[5])
                if b_cur:
                    order.append(b_cur[2])
                for f in order:
                    f()

            # ---- head: slots 0..1 ----------------------------------------
            xc0 = xc_pool.tile([97, BC], bf16, tag="xc")
            nc.sync.dma_start(xc0[:], xcat[0])
            emit_slot(a_phases(0, xc0), None, None)
            xc1 = xc_pool.tile([97, BC], bf16, tag="xc")
            nc.sync.dma_start(xc1[:], xcat[1])
            emit_slot(a_phases(1, xc1), None, b_phases(1))

            # ---- main loop: slots 2..505 ---------------------------------
            _hint = [mybir.EngineType.PE, mybir.EngineType.DVE,
                     mybir.EngineType.Activation, mybir.EngineType.Pool,
                     mybir.EngineType.SP]
            with tc.For_i(LOOP_LO, LOOP_HI, unroll, hint_engines=_hint) as iv:
                xslab = xcat[ds(iv, unroll)]
                xcu = []
                for u in range(unroll):
                    t_ = xc_pool.tile([97, BC], bf16, tag="xc")
                    nc.sync.dma_start(t_[:], xslab[u])
                    xcu.append(t_)
                for u in range(unroll):
                    tau = LOOP_LO + u  # slot = iv+u; mod-4 matches since iv%8==2
                    emit_slot(a_phases(tau, xcu[u]), b_phases(tau - 1), b_phases(tau))

            # ---- tail: slots 506..513 ------------------------------------
            for tau in range(LOOP_HI, T_run + 2):
                do_a = tau < T_run
                if do_a:
                    xct = xc_pool.tile([97, BC], bf16, tag="xc")
                    nc.sync.dma_start(xct[:], xcat[tau])
                    ap = a_phases(tau, xct)
                else:
                    ap = None
                bp_prev = b_phases(tau - 1) if tau - 1 <= T_run else None
                bp_cur = b_phases(tau) if tau <= T_run else None
                emit_slot(ap, bp_prev, bp_cur)

            nc.sync.dma_start(hout[:], rings[1][(T_run - 1) % 4][:])

    nc.compile()
    return nc


# ---------------------------------------------------------------------------
# Host-side weight prep
# ---------------------------------------------------------------------------

def _prep_weights(inputs):
    import ml_dtypes
    bf = ml_dtypes.bfloat16
    W = {k: np.asarray(v, np.float32) for k, v in inputs.items()}
    out = {}

    def pack_kt(wT):  # [256, 256] -> [128, 512] (kt, m)
        return np.concatenate([wT[0:128, :], wT[128:256, :]], axis=1)

    for l in range(L):
        fin = FS + FC if l == 0 else H
        Wg, Win, Wrec = W[f'Wg{l}'], W[f'Win{l}'], W[f'Wrec{l}']
        bg, tau = W[f'bg{l}'], W[f'tau{l}']
        itau = (1.0 / (softplus_np(tau) + 1.0)).astype(np.float32)
        WgxT = Wg[:, :fin].T
        WghT = Wg[:, fin:].T
        WrecT = Wrec.T
        WinT = Win.T
        dn = np.zeros((128, 256), np.float32)
        dnh = np.zeros((128, 256), np.float32)
        for m in range(2):
            dn[:, m * 128:(m + 1) * 128] = np.diag(-itau[m * 128:(m + 1) * 128])
            dnh[:, m * 128:(m + 1) * 128] = np.diag(-0.5 * itau[m * 128:(m + 1) * 128])
        if l == 0:
            out["g_x0"] = np.concatenate([WgxT, bg[None, :]], 0).astype(bf)
            out["in_x0"] = WinT.astype(bf)
            out["g_h0"] = pack_kt(WghT).astype(bf)
            out["g_h0h"] = pack_kt(WghT * 0.5).astype(bf)
            out["rec0"] = pack_kt(WrecT).astype(bf)
            out["rec0h"] = pack_kt(WrecT * 0.5).astype(bf)
            out["dneg0"] = dn.astype(bf)
            out["dneg0h"] = dnh.astype(bf)
        else:
            out["g_x1"] = pack_kt(WgxT).astype(bf)
            out["bg1row"] = bg[None, :].astype(bf)
            out["in_x1"] = pack_kt(WinT).astype(bf)
            out["g_h1"] = pack_kt(WghT).astype(bf)
            out["g_h1h"] = pack_kt(WghT * 0.5).astype(bf)
            out["rec1"] = pack_kt(WrecT).astype(bf)
            out["rec1h"] = pack_kt(WrecT * 0.5).astype(bf)
            out["dneg1"] = dn.astype(bf)
            out["dneg1h"] = dnh.astype(bf)
    lnT = np.zeros((2, 512), np.float32)
    lngN = np.zeros((1, 512), np.float32)
    for l in range(L):
        lng, lnb = W[f'lng{l}'], W[f'lnb{l}']
        for m in range(2):
            lq = (2 * l + m) * 128
            lnT[0, lq:lq + 128] = -lng[m * 128:(m + 1) * 128]
            lnT[1, lq:lq + 128] = lnb[m * 128:(m + 1) * 128]
            lngN[0, lq:lq + 128] = lng[m * 128:(m + 1) * 128] * N_H
    out["lnT"] = lnT.astype(bf)
    out["lngN"] = lngN.astype(bf)
    out["ones_red"] = np.ones((128, 1), np.float32).astype(bf)
    out["id6"] = (6.0 * np.eye(128, dtype=np.float32)).astype(bf)
    out["id1"] = np.eye(128, dtype=np.float32).astype(bf)
    out["id2"] = (2.0 * np.eye(128, dtype=np.float32)).astype(bf)
    out["ones2"] = np.ones((2, 64), np.float32).astype(bf)
    return out


def _prep_core_inputs(inputs, wpack, core, T_run=T):
    seq = np.asarray(inputs['seq_features'], np.float32)
    ctx = np.asarray(inputs['context_features'], np.float32)
    bsl = slice(core * BC, (core + 1) * BC)
    import ml_dtypes
    xc = np.empty((T_run, 97, BC), np.float32)
    xc[:, 0:64, :] = seq[bsl, :T_run].transpose(1, 2, 0)
    xc[:, 64:96, :] = ctx[bsl].T[None, :, :]
    xc[:, 96, :] = 1.0
    m = {"xcat": xc.astype(ml_dtypes.bfloat16)}
    m.update(wpack)
    return m


def _head(inputs, h1):
    cW1 = np.asarray(inputs['cW1'], np.float32)
    cb1 = np.asarray(inputs['cb1'], np.float32)
    cW2 = np.asarray(inputs['cW2'], np.float32)
    cb2 = np.asarray(inputs['cb2'], np.float32)
    hid = np.maximum(h1 @ cW1.T + cb1, 0)
    return (hid @ cW2.T + cb2).squeeze(-1)


_CACHE = {}


def kernel(**inputs):
    if "nc" not in _CACHE:
        _CACHE["nc"] = build_module(T, UNROLL)
    nc = _CACHE["nc"]
    from concourse.bass_utils import run_bass_kernel_spmd
    wpack = _prep_weights(inputs)
    in_maps = [_prep_core_inputs(inputs, wpack, c) for c in range(NCORES)]
    do_trace = os.environ.get("BASS_KERNEL_TRACE") == "1"
    r = run_bass_kernel_spmd(nc, in_maps, list(range(NCORES)), trace=do_trace)
    res = r.results
    if do_trace:
        _CACHE["exec_ns"] = r.exec_time_ns
        if r.instructions_and_trace is not None:
            _CACHE["trace_path"] = r.instructions_and_trace[1]
    h1 = np.empty((B, H), np.float32)
    for c in range(NCORES):
        ht = np.asarray(res[c]["hout"], np.float32)  # [128, (m,b)]
        bsl = slice(c * BC, (c + 1) * BC)
        for m in range(2):
            h1[bsl, m * 128:(m + 1) * 128] = ht[:, m * 64:(m + 1) * 64].T
    return _head(inputs, h1).astype(np.float32)


if __name__ == "__main__":
    pass


# revision 18
# speedup vs baseline: 1.2245x; 1.0114x over previous
"""Trainium2 Bass kernel for nn_LiquidNeuralNetwork (v2).

Strategy: data-parallel over batch (8 cores x 64). Per core, a fully on-chip
recurrence over T=512 steps with the two LTC layers run as TWO INDEPENDENT
INSTRUCTION STREAMS (layer0 at tick t, layer1 at tick t-1), interleaved at
RK4-stage granularity so the tensor-engine matmuls of one stream hide the
scalar/vector latency of the other.

Per-layer tiles are [128 part = h%128, free = m*64 + b] (m = h//128 output
half, b = batch-in-core). The gate sigmoid(tanh(u)) is replaced by the fitted
a*tanh(b*u)+0.5 (max abs err 6.7e-4) so each RK4 stage costs one ScalarE tanh
plus one fused DVE (t*a+0.5)*R multiply. The LayerNorm rsqrt runs as 4 custom
DVE row ops (linear-seed + reciprocal_approx_fast + 2 Newton steps in w-form).
RK4 P-assembly and dr-deltas run on the otherwise-idle GpSimd engine.
"""

import os
import sys
import numpy as np

sys.path.insert(0, "/opt/trn_rl_repo")

B, T, FS, FC, H, L = 512, 512, 64, 32, 256, 2
LN_EPS = 1e-5
NCORES = 8
BC = B // NCORES       # 64 batch per core
UNROLL = 8
N_H = 256.0

# gate fit: sigmoid(tanh(u)) ~= GATE_A * tanh(GATE_B * u) + 0.5
GATE_A, GATE_B = 0.230386, 1.072557

# rsqrt(vv) over observed vv range [1.0e5, 1.7e6] with 4x safety margin
VLO, VHI = 2.5e4, 6.8e6
_VC = float(np.sqrt(VLO * VHI))
SEED_B = 0.5 / float(np.sqrt(_VC))
SEED_A = 1.0 / (4.0 * SEED_B)
# w = SEED_B*(N*S2 + 36*N^2*eps - S1^2) + SEED_A ~= sqrt(vv)
VVW_C0 = SEED_B * N_H
VVW_C1 = SEED_A + SEED_B * 36.0 * N_H * N_H * LN_EPS
VVW_C2 = SEED_B
# Newton in w-form: y' = (1.5 - w*y^2*(0.5/b) + (0.5a/b)*y^2) * y
NRW_C0 = 1.5
NRW_C1 = 0.5 / SEED_B
NRW_C2 = 0.5 * SEED_A / SEED_B

USE_GPSIMD = os.environ.get("LNN_NO_GPSIMD") != "1"


def softplus_np(x):
    return np.log1p(np.exp(-np.abs(x))) + np.maximum(x, 0)


# ---------------------------------------------------------------------------
# Custom DVE ops
# ---------------------------------------------------------------------------

_OPS_CACHE = {}


def _get_custom_ops():
    if _OPS_CACHE:
        return _OPS_CACHE
    from concourse.dve_spec import Spec, Src0, Src1, C0, C1, C2, lower, sq
    from concourse.dve_spec import _has_src1
    from concourse.dve_uop import DveOpSpec
    from concourse import dve_ops

    _m = sq(Src1)
    defs = {
        # out = (in0*s0 + s1) * in1   -- gate affine folded into the R-multiply
        "GATE_MUL_LNN": (
            (Src0 * C0 + C1) * Src1,
            lambda in0, in1, s0, s1, imm2: (
                (in0.astype(np.float32) * s0 + s1) * in1
            ).astype(np.float32),
        ),
        # out = (in0*s0 + s1) - in1^2 * imm2   -- w = b*(N*S2 + c - S1^2) + a
        "VV_W_LNN": (
            (Src0 * C0 + C1) - sq(Src1) * C2,
            lambda in0, in1, s0, s1, imm2: (
                (in0.astype(np.float32) * s0 + s1)
                - np.square(in1.astype(np.float32)) * imm2
            ).astype(np.float32),
        ),
        # out = (s0 - in0*in1^2*s1 + imm2*in1^2) * in1  -- Newton step, w-form
        "NR_W_LNN": (
            (C0 - Src0 * _m * C1 + C2 * _m) * Src1,
            lambda in0, in1, s0, s1, imm2: (
                (
                    s0
                    - in0.astype(np.float32) * np.square(in1.astype(np.float32)) * s1
                    + imm2 * np.square(in1.astype(np.float32))
                )
                * in1
            ).astype(np.float32),
        ),
    }
    for name, (body, ref) in defs.items():
        if name in dve_ops._SUB_OPCODE_FOR_NAME:
            _OPS_CACHE[name] = next(o for o in dve_ops.OPS if o.name == name)
            continue
        spec = Spec(body=body, reference=ref)
        opcode = dve_ops._CUSTOM_DVE_ROW_BASE + len(dve_ops.OPS)
        shas = {}
        for ver in ("v3", "v4"):
            shas[ver] = DveOpSpec(
                name=name,
                opcode=opcode,
                uops=lower(spec, ver=ver),
                rd1_en=_has_src1(spec),
            ).sha(ver)
        op = dve_ops.DveOp(name, spec, subdim=False, uops_sha=shas)
        dve_ops.OPS.append(op)
        dve_ops._SUB_OPCODE_FOR_NAME[name] = opcode
        dve_ops.CUSTOM_DVE_SPECS[name] = spec
        _OPS_CACHE[name] = op
    return _OPS_CACHE


# ---------------------------------------------------------------------------
# Bass module builder
# ---------------------------------------------------------------------------

def build_module(T_run=T, unroll=UNROLL):
    import concourse.bass as bass
    import concourse.mybir as mybir
    from concourse import tile, bacc
    from concourse.bass import ds

    ops = _get_custom_ops()
    GATE_MUL = ops["GATE_MUL_LNN"]
    VV_W = ops["VV_W_LNN"]
    NR_W = ops["NR_W_LNN"]

    f32 = mybir.dt.float32
    bf16 = mybir.dt.bfloat16
    AF = mybir.ActivationFunctionType
    OP = mybir.AluOpType

    nc = bacc.Bacc(None, target_bir_lowering=False)

    xcat = nc.declare_dram_parameter("xcat", [T_run, 97, BC], bf16, isOutput=False)
    wdecl = {}

    def wparam(name, shape):
        wdecl[name] = nc.declare_dram_parameter(name, shape, bf16, isOutput=False)
        return wdecl[name]

    wparam("g_x0", [97, 256])
    wparam("in_x0", [96, 256])
    wparam("g_h0", [128, 512])
    wparam("g_h0h", [128, 512])
    wparam("rec0", [128, 512])
    wparam("rec0h", [128, 512])
    wparam("dneg0", [128, 256])
    wparam("dneg0h", [128, 256])
    wparam("g_x1", [128, 512])
    wparam("bg1row", [1, 256])
    wparam("in_x1", [128, 512])
    wparam("g_h1", [128, 512])
    wparam("g_h1h", [128, 512])
    wparam("rec1", [128, 512])
    wparam("rec1h", [128, 512])
    wparam("dneg1", [128, 256])
    wparam("dneg1h", [128, 256])
    wparam("lnT", [2, 512])      # rows: [-lng ; lnb], col blocks (2l+m)*128
    wparam("lngN", [1, 512])     # lng * N
    wparam("ones_red", [128, 1])
    wparam("id6", [128, 128])
    wparam("id1", [128, 128])
    wparam("id2", [128, 128])
    wparam("ones2", [2, 64])
    hout = nc.declare_dram_parameter("hout", [128, 128], bf16, isOutput=True)

    # slots 2..505 in the hw loop; 0..1 head; 506..513 tail
    LOOP_LO, LOOP_HI = 2, 506
    assert (LOOP_HI - LOOP_LO) % unroll == 0

    with tile.TileContext(nc) as tc:
        from contextlib import ExitStack
        with ExitStack() as ctx:
            singles = ctx.enter_context(tc.tile_pool(name="singles", bufs=1))
            xc_pool = ctx.enter_context(tc.tile_pool(name="xc", bufs=12))
            # PSUM: one bank each for G/R/V per layer; bc+stats share a bank.
            gps = [ctx.enter_context(tc.tile_pool(name=f"g{l}ps", bufs=1, space="PSUM")) for l in range(L)]
            rps = [ctx.enter_context(tc.tile_pool(name=f"r{l}ps", bufs=1, space="PSUM")) for l in range(L)]
            vps = [ctx.enter_context(tc.tile_pool(name=f"v{l}ps", bufs=1, space="PSUM")) for l in range(L)]
            tailps = [ctx.enter_context(tc.tile_pool(name=f"tail{l}ps", bufs=1, space="PSUM")) for l in range(L)]

            # ---- resident weights ---------------------------------------
            W = {}
            for name, dram in wdecl.items():
                t_ = singles.tile(list(dram.shape), bf16, name=name, tag=name)
                nc.sync.dma_start(t_[:], dram[:])
                W[name] = t_

            rings = [[singles.tile([128, 128], bf16, name=f"ring{l}_{i}", tag=f"ring{l}_{i}")
                      for i in range(4)] for l in range(L)]
            hz = singles.tile([128, 128], bf16)
            nc.vector.memset(hz[:], 0.0)
            nc.vector.memset(rings[1][3][:], 0.0)   # h1[-1] = 0
            onesr = singles.tile([1, BC], bf16)
            nc.vector.memset(onesr[:], 1.0)
            trhs = [singles.tile([2, BC], bf16, name=f"trhs{l}", tag=f"trhs{l}") for l in range(L)]
            for l in range(L):
                nc.sync.dma_start(trhs[l][:, :], wdecl["ones2"][:, :])

            # static per-stream work tiles (fixed names -> the software
            # pipeline can reference them across the hw-loop back edge)
            def mk(l, nm, dt):
                return singles.tile([128, 128], dt, name=f"{nm}{l}", tag=f"{nm}{l}")

            ST = []
            for l in range(L):
                d = dict(
                    tg=mk(l, "tg", f32), mm=mk(l, "mm", f32),
                    k1=mk(l, "k1", bf16), k2=mk(l, "k2", bf16),
                    k3=mk(l, "k3", bf16), k4=mk(l, "k4", bf16),
                    dr2=mk(l, "dr2", bf16), dr3=mk(l, "dr3", bf16),
                    P=mk(l, "P", bf16), P2=mk(l, "P2", bf16),
                    z=mk(l, "z", f32), z2=mk(l, "z2", f32),
                )
                for nm in ("sx", "w", "y0", "y1"):
                    d[nm] = singles.tile([1, BC], f32, name=nm + str(l), tag=nm + str(l))
                d["y2"] = singles.tile([1, BC], bf16, name=f"y2{l}", tag=f"y2{l}")
                d["G"] = gps[l].tile([128, 128], f32, name=f"G{l}", tag="G")
                d["R"] = rps[l].tile([128, 128], f32, name=f"R{l}", tag="R")
                d["V"] = vps[l].tile([128, 128], f32, name=f"V{l}", tag="V")
                d["tail"] = tailps[l].tile([128, 512], f32, name=f"tail{l}", tag="tail")
                ST.append(d)

            def wt(name, kt, m):
                return W[name][:, kt * 256 + m * 128: kt * 256 + (m + 1) * 128]

            def dneg(l, half, m):
                nm = f"dneg{l}" + ("h" if half else "")
                return W[nm][:, m * 128:(m + 1) * 128]

            eng2 = nc.gpsimd if USE_GPSIMD else nc.vector

            def g_sub(out, a, b):  # out = a - b  (SBUF-only operands)
                if USE_GPSIMD:
                    eng2.tensor_sub(out, a, b)
                else:
                    nc.vector.scalar_tensor_tensor(out=out, in0=b, scalar=-1.0,
                                                   in1=a, op0=OP.mult, op1=OP.add)

            def g_add(out, a, b):
                if USE_GPSIMD:
                    eng2.tensor_add(out, a, b)
                else:
                    nc.vector.scalar_tensor_tensor(out=out, in0=a, scalar=0.0,
                                                   in1=b, op0=OP.add, op1=OP.add)

            def g_mul(out, a, b):
                if USE_GPSIMD:
                    eng2.tensor_mul(out, a, b)
                else:
                    nc.vector.scalar_tensor_tensor(out=out, in0=a, scalar=0.0,
                                                   in1=b, op0=OP.add, op1=OP.mult)

            # per-stream bookkeeping (mm-group first flags survive one tick)
            first_flags = [{}, {}]

            def emit_stage(l, s, xc=None, x1=None, h_self=None):
                """RK4 stage s for layer l: G MMs, gate tanh, R/V MMs, fused
                gate multiply, k, and the next stage's dr."""
                d = ST[l]
                first = first_flags[l]

                def Gc(m):
                    return d["G"][:, m * 64:(m + 1) * 64]

                def Rc(m):
                    return d["R"][:, m * 64:(m + 1) * 64]

                def Vc(m):
                    return d["V"][:, m * 64:(m + 1) * 64]

                def bmm(bank, out_ap, lhsT, rhs, last=False):
                    st_ = bank not in first
                    first[bank] = True
                    nc.tensor.matmul(out_ap, lhsT, rhs, start=st_, stop=last,
                                     skip_group_check=True)

                Pp = d["tail"][:, 384:512]
                dr = {1: None, 2: d["k1"], 3: d["dr2"], 4: d["dr3"]}[s]
                if s == 1:
                    first.clear()
                    nc.tensor.matmul(Pp, W["id6"][:, :], h_self[:, :],
                                     start=True, stop=False, skip_group_check=True)
                    if l == 0:
                        for m in range(2):
                            bmm("G", Gc(m), W["g_x0"][:, m * 128:(m + 1) * 128], xc[:, :])
                            for kt in range(2):
                                bmm("G", Gc(m), wt("g_h0", kt, m), h_self[:, kt * 64:(kt + 1) * 64])
                    else:
                        for m in range(2):
                            for kt in range(2):
                                bmm("G", Gc(m), wt("g_x1", kt, m), x1[:, kt * 64:(kt + 1) * 64])
                            bmm("G", Gc(m), W["bg1row"][:, m * 128:(m + 1) * 128], onesr[:, :])
                            for kt in range(2):
                                bmm("G", Gc(m), wt("g_h1", kt, m), h_self[:, kt * 64:(kt + 1) * 64])
                    nc.scalar.activation(d["tg"][:, :], d["G"][:, :], AF.Tanh, scale=GATE_B)
                    for m in range(2):
                        for kt in range(2):
                            bmm("R", Rc(m), wt(f"rec{l}", kt, m), h_self[:, kt * 64:(kt + 1) * 64])
                    if l == 0:
                        for m in range(2):
                            bmm("V", Vc(m), W["in_x0"][:, m * 128:(m + 1) * 128], xc[0:96, :])
                            bmm("V", Vc(m), dneg(0, False, m), h_self[:, m * 64:(m + 1) * 64])
                    else:
                        for m in range(2):
                            for kt in range(2):
                                bmm("V", Vc(m), wt("in_x1", kt, m), x1[:, kt * 64:(kt + 1) * 64])
                            bmm("V", Vc(m), dneg(1, False, m), h_self[:, m * 64:(m + 1) * 64])
                else:
                    half = s in (2, 3)
                    sfx = "h" if half else ""
                    last = s == 4
                    for m in range(2):
                        for kt in range(2):
                            bmm("G", Gc(m), wt(f"g_h{l}{sfx}", kt, m),
                                dr[:, kt * 64:(kt + 1) * 64], last=last and m == 1 and kt == 1)
                    nc.scalar.activation(d["tg"][:, :], d["G"][:, :], AF.Tanh, scale=GATE_B)
                    for m in range(2):
                        for kt in range(2):
                            bmm("R", Rc(m), wt(f"rec{l}{sfx}", kt, m),
                                dr[:, kt * 64:(kt + 1) * 64], last=last and m == 1 and kt == 1)
                    for m in range(2):
                        bmm("V", Vc(m), dneg(l, half, m),
                            dr[:, m * 64:(m + 1) * 64], last=last and m == 1)
                if s >= 2:
                    # accumulate the previous stage's k into P (identity MMs)
                    kprev = d[f"k{s-1}"]
                    idw = "id1" if s == 2 else "id2"
                    nc.tensor.matmul(Pp, W[idw][:, :], kprev[:, :],
                                     start=False, stop=False, skip_group_check=True)
                nc.vector._custom_dve(GATE_MUL, out=d["mm"][:, :], in0=d["tg"][:, :],
                                      in1=d["R"][:, :], s0=GATE_A, s1=0.5)
                kk = d[f"k{s}"]
                nc.vector.scalar_tensor_tensor(out=kk[:, :], in0=d["mm"][:, :], scalar=0.0,
                                               in1=d["V"][:, :], op0=OP.add, op1=OP.add)
                if s == 2:
                    nc.vector.scalar_tensor_tensor(out=d["dr2"][:, :], in0=d["k1"][:, :],
                                                   scalar=-1.0, in1=kk[:, :],
                                                   op0=OP.mult, op1=OP.add)
                elif s == 3:
                    nc.vector.scalar_tensor_tensor(out=d["dr3"][:, :], in0=d["k2"][:, :],
                                                   scalar=-0.5, in1=kk[:, :],
                                                   op0=OP.mult, op1=OP.add)

            def emit_tail_a(l, h_self):
                d = ST[l]
                tl = d["tail"]
                Pp = tl[:, 384:512]
                nc.tensor.matmul(Pp, W["id1"][:, :], d["k4"][:, :],
                                 start=False, stop=True, skip_group_check=True)
                nc.vector.tensor_scalar(out=d["P"][:, :], in0=Pp, scalar1=1.0,
                                        scalar2=None, op0=OP.mult)
                nc.scalar.activation(d["P2"][:, :], Pp, AF.Square)
                nc.tensor.matmul(tl[0:1, 256:320], W["ones_red"][:, :], d["P"][:, 0:64],
                                 start=True, stop=False, skip_group_check=True)
                nc.tensor.matmul(tl[0:1, 256:320], W["ones_red"][:, :], d["P"][:, 64:128],
                                 start=False, stop=True, skip_group_check=True)
                nc.vector.tensor_scalar(out=d["sx"][:, :], in0=tl[0:1, 256:320], scalar1=1.0,
                                        scalar2=None, op0=OP.mult)
                nc.tensor.matmul(tl[0:1, 320:384], W["ones_red"][:, :], d["P2"][:, 0:64],
                                 start=True, stop=False, skip_group_check=True)
                nc.tensor.matmul(tl[0:1, 320:384], W["ones_red"][:, :], d["P2"][:, 64:128],
                                 start=False, stop=True, skip_group_check=True)

            def emit_tail_b(l, h_next):
                d = ST[l]
                tl = d["tail"]
                s2_ap = tl[0:1, 320:384]
                nc.vector._custom_dve(VV_W, out=d["w"][:, :], in0=s2_ap,
                                      in1=d["sx"][:, :], s0=VVW_C0, s1=VVW_C1, imm2=VVW_C2)
                nc.vector.reciprocal_approx_fast(out=d["y0"][:, :], in_=d["w"][:, :])
                nc.vector._custom_dve(NR_W, out=d["y1"][:, :], in0=d["w"][:, :], in1=d["y0"][:, :],
                                      s0=NRW_C0, s1=NRW_C1, imm2=NRW_C2)
                nc.vector._custom_dve(NR_W, out=d["y2"][:, :], in0=d["w"][:, :], in1=d["y1"][:, :],
                                      s0=NRW_C0, s1=NRW_C1, imm2=NRW_C2)
                nc.vector.scalar_tensor_tensor(out=trhs[l][0:1, :], in0=d["sx"][:, :],
                                               scalar=0.0, in1=d["y2"][:, :],
                                               op0=OP.add, op1=OP.mult)
                for m in range(2):
                    lq = (2 * l + m) * 128
                    nc.tensor.matmul(tl[:, m * 64:(m + 1) * 64], W["lngN"][:, lq:lq + 128],
                                     d["y2"][0:1, :], start=True, stop=True, skip_group_check=True)
                for m in range(2):
                    lq = (2 * l + m) * 128
                    nc.tensor.matmul(tl[:, 128 + m * 64:128 + (m + 1) * 64], W["lnT"][:, lq:lq + 128],
                                     trhs[l][:, :], start=True, stop=True, skip_group_check=True)
                nc.vector.scalar_tensor_tensor(out=d["z"][:, :], in0=d["P"][:, :], scalar=0.0,
                                               in1=tl[:, 0:128], op0=OP.add, op1=OP.mult)
                nc.vector.scalar_tensor_tensor(out=d["z2"][:, :], in0=d["z"][:, :], scalar=0.0,
                                               in1=tl[:, 128:256], op0=OP.add, op1=OP.add)
                nc.scalar.activation(h_next[:, :], d["z2"][:, :], AF.Tanh)

            r0, r1 = rings[0], rings[1]

            def a_phases(tau, xc):
                """A = layer0 tick tau: [s1, s2, s3, s4, tail_a, tail_b]."""
                hs = hz if tau == 0 else r0[(tau - 1) % 4]
                return [
                    lambda: emit_stage(0, 1, xc=xc, h_self=hs),
                    lambda: emit_stage(0, 2, h_self=hs),
                    lambda: emit_stage(0, 3, h_self=hs),
                    lambda: emit_stage(0, 4, h_self=hs),
                    lambda: emit_tail_a(0, hs),
                    lambda: emit_tail_b(0, r0[tau % 4]),
                ]

            def b_phases(tau):
                """B = layer1 tick tau-1 (emitted during slots tau/tau+1).
                For tau==1, r1[3] is pre-zeroed and serves as h1[-1]."""
                hs = r1[(tau - 2) % 4]
                x1 = r0[(tau - 1) % 4]
                return [
                    lambda: emit_stage(1, 1, x1=x1, h_self=hs),
                    lambda: emit_stage(1, 2, h_self=hs),
                    lambda: emit_stage(1, 3, h_self=hs),
                    lambda: emit_stage(1, 4, h_self=hs),
                    lambda: emit_tail_a(1, hs),
                    lambda: emit_tail_b(1, r1[(tau - 1) % 4]),
                ]

            def emit_slot(a_ph, b_prev, b_cur):
                """Steady-state slot: A's 6 phases; B(prev slot) finishes its
                last 3 phases under A's first stages, B(cur) starts its first
                3 under A's tail."""
                order = []
                if a_ph:
                    order.append(a_ph[0])
                if b_prev:
                    order.append(b_prev[3])
                if a_ph:
                    order.append(a_ph[1])
                if b_prev:
                    order.append(b_prev[4])
                if a_ph:
                    order.append(a_ph[2])
                if b_prev:
                    order.append(b_prev[5])
                if a_ph:
                    order.append(a_ph[3])
                if b_cur:
                    order.append(b_cur[0])
                if a_ph:
                    order.append(a_ph[4])
                if b_cur:
                    order.append(b_cur[1])
                if a_ph:
                    order.append(a_ph[5])
                if b_cur:
                    order.append(b_cur[2])
                for f in order:
                    f()

            # ---- head: slots 0..1 ----------------------------------------
            xc0 = xc_pool.tile([97, BC], bf16, tag="xc")
            nc.sync.dma_start(xc0[:], xcat[0])
            emit_slot(a_phases(0, xc0), None, None)
            xc1 = xc_pool.tile([97, BC], bf16, tag="xc")
            nc.sync.dma_start(xc1[:], xcat[1])
            emit_slot(a_phases(1, xc1), None, b_phases(1))

            # ---- main loop: slots 2..505 ---------------------------------
            _hint = [mybir.EngineType.PE, mybir.EngineType.DVE,
                     mybir.EngineType.Activation, mybir.EngineType.Pool,
                     mybir.EngineType.SP]
            with tc.For_i(LOOP_LO, LOOP_HI, unroll, hint_engines=_hint) as iv:
                xslab = xcat[ds(iv, unroll)]
                xcu = []
                for u in range(unroll):
                    t_ = xc_pool.tile([97, BC], bf16, tag="xc")
                    nc.sync.dma_start(t_[:], xslab[u])
                    xcu.append(t_)
                for u in range(unroll):
                    tau = LOOP_LO + u  # slot = iv+u; mod-4 matches since iv%8==2
                    emit_slot(a_phases(tau, xcu[u]), b_phases(tau - 1), b_phases(tau))

            # ---- tail: slots 506..513 ------------------------------------
            for tau in range(LOOP_HI, T_run + 2):
                do_a = tau < T_run
                if do_a:
                    xct = xc_pool.tile([97, BC], bf16, tag="xc")
                    nc.sync.dma_start(xct[:], xcat[tau])
                    ap = a_phases(tau, xct)
                else:
                    ap = None
                bp_prev = b_phases(tau - 1) if tau - 1 <= T_run else None
                bp_cur = b_phases(tau) if tau <= T_run else None
                emit_slot(ap, bp_prev, bp_cur)

            nc.sync.dma_start(hout[:], rings[1][(T_run - 1) % 4][:])

    nc.compile()
    return nc


# ---------------------------------------------------------------------------
# Host-side weight prep
# ---------------------------------------------------------------------------

def _prep_weights(inputs):
    import ml_dtypes
    bf = ml_dtypes.bfloat16
    W = {k: np.asarray(v, np.float32) for k, v in inputs.items()}
    out = {}

    def pack_kt(wT):  # [256, 256] -> [128, 512] (kt, m)
        return np.concatenate([wT[0:128, :], wT[128:256, :]], axis=1)

    for l in range(L):
        fin = FS + FC if l == 0 else H
        Wg, Win, Wrec = W[f'Wg{l}'], W[f'Win{l}'], W[f'Wrec{l}']
        bg, tau = W[f'bg{l}'], W[f'tau{l}']
        itau = (1.0 / (softplus_np(tau) + 1.0)).astype(np.float32)
        WgxT = Wg[:, :fin].T
        WghT = Wg[:, fin:].T
        WrecT = Wrec.T
        WinT = Win.T
        dn = np.zeros((128, 256), np.float32)
        dnh = np.zeros((128, 256), np.float32)
        for m in range(2):
            dn[:, m * 128:(m + 1) * 128] = np.diag(-itau[m * 128:(m + 1) * 128])
            dnh[:, m * 128:(m + 1) * 128] = np.diag(-0.5 * itau[m * 128:(m + 1) * 128])
        if l == 0:
            out["g_x0"] = np.concatenate([WgxT, bg[None, :]], 0).astype(bf)
            out["in_x0"] = WinT.astype(bf)
            out["g_h0"] = pack_kt(WghT).astype(bf)
            out["g_h0h"] = pack_kt(WghT * 0.5).astype(bf)
            out["rec0"] = pack_kt(WrecT).astype(bf)
            out["rec0h"] = pack_kt(WrecT * 0.5).astype(bf)
            out["dneg0"] = dn.astype(bf)
            out["dneg0h"] = dnh.astype(bf)
        else:
            out["g_x1"] = pack_kt(WgxT).astype(bf)
            out["bg1row"] = bg[None, :].astype(bf)
            out["in_x1"] = pack_kt(WinT).astype(bf)
            out["g_h1"] = pack_kt(WghT).astype(bf)
            out["g_h1h"] = pack_kt(WghT * 0.5).astype(bf)
            out["rec1"] = pack_kt(WrecT).astype(bf)
            out["rec1h"] = pack_kt(WrecT * 0.5).astype(bf)
            out["dneg1"] = dn.astype(bf)
            out["dneg1h"] = dnh.astype(bf)
    lnT = np.zeros((2, 512), np.float32)
    lngN = np.zeros((1, 512), np.float32)
    for l in range(L):
        lng, lnb = W[f'lng{l}'], W[f'lnb{l}']
        for m in range(2):
            lq = (2 * l + m) * 128
            lnT[0, lq:lq + 128] = -lng[m * 128:(m + 1) * 128]
            lnT[1, lq:lq + 128] = lnb[m * 128:(m + 1) * 128]
            lngN[0, lq:lq + 128] = lng[m * 128:(m + 1) * 128] * N_H
    out["lnT"] = lnT.astype(bf)
    out["lngN"] = lngN.astype(bf)
    out["ones_red"] = np.ones((128, 1), np.float32).astype(bf)
    out["id6"] = (6.0 * np.eye(128, dtype=np.float32)).astype(bf)
    out["id1"] = np.eye(128, dtype=np.float32).astype(bf)
    out["id2"] = (2.0 * np.eye(128, dtype=np.float32)).astype(bf)
    out["ones2"] = np.ones((2, 64), np.float32).astype(bf)
    return out


def _prep_core_inputs(inputs, wpack, core, T_run=T):
    seq = np.asarray(inputs['seq_features'], np.float32)
    ctx = np.asarray(inputs['context_features'], np.float32)
    bsl = slice(core * BC, (core + 1) * BC)
    import ml_dtypes
    xc = np.empty((T_run, 97, BC), np.float32)
    xc[:, 0:64, :] = seq[bsl, :T_run].transpose(1, 2, 0)
    xc[:, 64:96, :] = ctx[bsl].T[None, :, :]
    xc[:, 96, :] = 1.0
    m = {"xcat": xc.astype(ml_dtypes.bfloat16)}
    m.update(wpack)
    return m


def _head(inputs, h1):
    cW1 = np.asarray(inputs['cW1'], np.float32)
    cb1 = np.asarray(inputs['cb1'], np.float32)
    cW2 = np.asarray(inputs['cW2'], np.float32)
    cb2 = np.asarray(inputs['cb2'], np.float32)
    hid = np.maximum(h1 @ cW1.T + cb1, 0)
    return (hid @ cW2.T + cb2).squeeze(-1)


_CACHE = {}


def kernel(**inputs):
    if "nc" not in _CACHE:
        _CACHE["nc"] = build_module(T, UNROLL)
    nc = _CACHE["nc"]
    from concourse.bass_utils import run_bass_kernel_spmd
    wpack = _prep_weights(inputs)
    in_maps = [_prep_core_inputs(inputs, wpack, c) for c in range(NCORES)]
    do_trace = os.environ.get("BASS_KERNEL_TRACE") == "1"
    r = run_bass_kernel_spmd(nc, in_maps, list(range(NCORES)), trace=do_trace)
    res = r.results
    if do_trace:
        _CACHE["exec_ns"] = r.exec_time_ns
        if r.instructions_and_trace is not None:
            _CACHE["trace_path"] = r.instructions_and_trace[1]
    h1 = np.empty((B, H), np.float32)
    for c in range(NCORES):
        ht = np.asarray(res[c]["hout"], np.float32)  # [128, (m,b)]
        bsl = slice(c * BC, (c + 1) * BC)
        for m in range(2):
            h1[bsl, m * 128:(m + 1) * 128] = ht[:, m * 64:(m + 1) * 64].T
    return _head(inputs, h1).astype(np.float32)


if __name__ == "__main__":
    pass
